# revision 1
# baseline (speedup 1.0000x reference)
"""Trainium2 Bass kernel for nn_BasicBlock (dense transformer block).

Data-parallel over batch: B=8 batch elements, one per NeuronCore, no
collectives. Mixed fp8e4m3-DoubleRow / bf16 rewrite of the f32r baseline.

Per-core design (S=1024, D=1024, H=16, d_k=64, d_ff=4096):
  - Q/K projections run fp8e4m3 DoubleRow (K=256/instruction, 0.5 PE
    cycles/column); x arrives host-pre-transposed in both fp8 (Q/K) and
    bf16 (V) layouts. Numerics: fp8 on the V/out-proj/FFN paths measurably
    breaks the 2e-2 budget (causal early tokens amplify attention-path
    quantization), so V-proj, out-proj, FFN1 and FFN2 run bf16 at the full
    1 cycle/column PE rate with host-cast bf16 weights.
  - Attention: scoresT [s_k-part, s_q-free] in bf16 per (head, key-chunk),
    causal columns only, 4-deep PSUM rotation so exp latency never blocks
    the PE; exp on ScalarE straight out of PSUM into bf16 expT; strict
    lower-triangle mask as one diagonal-strided DVE multiply per head;
    BMM2 with a ones-augmented V (65th row accumulates the softmax
    denominator); reciprocal on DVE; denominator broadcast across 64
    partitions via a 1-row f32r PE matmul; normalize fused into the
    bf16 quantize multiply. Head pipeline is software-interleaved
    (scores / QK-proj halves / BMM2 / normalize) to keep PE fed.
  - Attention row 0 (zero-pad row in the reference) produces NaNs on
    device (0/0); the host recomputes output row 0 exactly in numpy.
  - FFN weights stream from DRAM (W1 host-pre-tiled, W2 prefetched 2
    chunks ahead); FFN2 runs in two 4-token-chunk passes so 8 PSUM banks
    hold the accumulators; h1 transposes pipeline one chunk behind the
    out-proj+LN1 loop.
  - Fast path assumes the reference's structural constants (biases 0,
    gammas 1, betas 0) and skips those ops; other inputs fall back to a
    numpy reference implementation.
"""

import numpy as np
import ml_dtypes
import concourse.bass as bass
import concourse.tile as tile
from concourse import bacc, mybir
from concourse.bass_utils import run_bass_kernel_spmd

F32 = mybir.dt.float32
F32R = mybir.dt.float32r
F8 = mybir.dt.float8e4
BF16 = mybir.dt.bfloat16
AF = mybir.ActivationFunctionType
OP = mybir.AluOpType
DR = mybir.MatmulPerfMode.DoubleRow

NP_F8 = ml_dtypes.float8_e4m3
NP_BF16 = ml_dtypes.bfloat16

B, S, D, H, DK, DFF = 8, 1024, 1024, 16, 64, 4096
P = 128
DC = D // P       # 8 chunks of d_model
FC = DFF // P     # 32 chunks of d_ff
SC = S // P       # 8 chunks of sequence
EPS = 1e-5
SCALE = 0.125     # 1/sqrt(DK)

FFN1_BF16 = True  # FFN1 in bf16 (precision), FFN2 stays fp8 DoubleRow


def _build():
    nc = bacc.Bacc("TRN2", target_bir_lowering=False, debug=False, num_devices=B)

    xT_d = nc.dram_tensor("xT8", [D, S], F8, kind="ExternalInput").ap()
    x_d = nc.dram_tensor("x32", [S, D], F32, kind="ExternalInput").ap()
    wq_d = nc.dram_tensor("wq8", [D, D], F8, kind="ExternalInput").ap()
    wk_d = nc.dram_tensor("wk8", [D, D], F8, kind="ExternalInput").ap()
    wv_d = nc.dram_tensor("wvb", [D, D], BF16, kind="ExternalInput").ap()
    wo_d = nc.dram_tensor("wob", [D, D], BF16, kind="ExternalInput").ap()
    xTb_d = nc.dram_tensor("xTb", [D, S], BF16, kind="ExternalInput").ap()
    W1DT = BF16 if FFN1_BF16 else F8
    # host pre-arranged: w1p[c, p, t, f] = W1[128t+p, 128c+f]
    w1_d = nc.dram_tensor("w1p", [FC, P, DC, P], W1DT, kind="ExternalInput").ap()
    w2_d = nc.dram_tensor("w2b", [DFF, D], BF16, kind="ExternalInput").ap()
    mask_d = nc.dram_tensor("mask8", [P, P], BF16, kind="ExternalInput").ap()
    id_d = nc.dram_tensor("ident", [P, P], F32, kind="ExternalInput").ap()
    ones_d = nc.dram_tensor("onesf", [P, 64], F32, kind="ExternalInput").ap()
    out_d = nc.dram_tensor("out", [S, D], F32, kind="ExternalOutput").ap()

    with tile.TileContext(nc) as tc:
      with tc.tile_pool(name="singles", bufs=1) as sg, \
           tc.tile_pool(name="sm", bufs=6) as sm:
        wo = sg.tile([P, DC, D], BF16)
        mask_rep = sg.tile([P, SC, P], BF16)
        ident = sg.tile([P, P], F32)
        ones1r = sg.tile([P, 64], F32R)
        eps_t = sg.tile([P, 1], F32)
        concatT = sg.tile([P, DC, S], BF16)
        h1 = sg.tile([P, SC, D], F32)
        h1T = sg.tile([P, DC, S], BF16 if FFN1_BF16 else F8)

        nc.vector.memset(eps_t[:], EPS)
        nc.sync.dma_start(ident[:], id_d)
        nc.sync.dma_start(ones1r[:], ones_d.bitcast(F32R))
        # mask replicated 8x along a middle dim for the diagonal-AP multiply
        mask_bc = bass.AP(tensor=mask_d.tensor, offset=mask_d.offset,
                          ap=[[P, P], [0, SC], [1, P]])
        nc.sync.dma_start(mask_rep[:], mask_bc)

        attp_cm = tc.tile_pool(name="attp", bufs=1)
        ap_ = attp_cm.__enter__()
        xT = ap_.tile([P, DC, S], F8)
        wq = ap_.tile([P, DC, D], F8)
        wk = ap_.tile([P, DC, D], F8)
        wv = ap_.tile([P, DC, D], BF16)
        xTb = ap_.tile([P, DC, S], BF16)
        vaug = ap_.tile([P, SC, H, DK + 1], BF16)

        # split x/wv loads so the first V matmuls can start at the half-way
        # point of the DMA stream
        xTb_r = xTb_d.rearrange("(c p) s -> p c s", p=P)
        wv_r = wv_d.rearrange("(c p) d -> p c d", p=P)
        nc.sync.dma_start(xTb[:, 0:4, :], xTb_r[:, 0:4, :])
        nc.sync.dma_start(wv[:, 0:4, :], wv_r[:, 0:4, :])
        nc.sync.dma_start(xTb[:, 4:8, :], xTb_r[:, 4:8, :])
        nc.sync.dma_start(wv[:, 4:8, :], wv_r[:, 4:8, :])
        nc.sync.dma_start(xT[:], xT_d.rearrange("(c p) s -> p c s", p=P))
        nc.sync.dma_start(wq[:], wq_d.rearrange("(c p) d -> p c d", p=P))
        nc.sync.dma_start(wk[:], wk_d.rearrange("(c p) d -> p c d", p=P))
        nc.sync.dma_start(wo[:], wo_d.rearrange("(c p) d -> p c d", p=P))
        nc.vector.memset(vaug[:, :, :, DK:DK + 1], 1.0)

        def ln_finish(stm, ap_full):
            # stm [P, 2, 6] bn_stats pairs; ap_full [P, 1024] in SBUF
            mv = sm.tile([P, 2], F32, tag="mv", name="mv")
            rs = sm.tile([P, 1], F32, tag="rs", name="rs")
            nb = sm.tile([P, 1], F32, tag="nb", name="nb")
            nc.vector.bn_aggr(mv[:], stm[:])
            nc.scalar.activation(rs[:], mv[:, 1:2], AF.Sqrt,
                                 bias=eps_t[:], scale=1.0)
            nc.vector.reciprocal(rs[:], rs[:])
            nc.vector.tensor_scalar(nb[:], mv[:, 0:1], rs[:], -1.0,
                                    op0=OP.mult, op1=OP.mult)
            nc.scalar.activation(ap_full, ap_full, AF.Identity,
                                 bias=nb[:], scale=rs[:])

        # ================= attention =================
        with tc.tile_pool(name="qkp", bufs=3) as qkp, \
             tc.tile_pool(name="expp", bufs=2) as expp, \
             tc.tile_pool(name="recp", bufs=1) as recp, \
             tc.tile_pool(name="psQK", bufs=1, space="PSUM") as psQK, \
             tc.tile_pool(name="psS", bufs=4, space="PSUM") as psS, \
             tc.tile_pool(name="psCD", bufs=1, space="PSUM") as psCD:

            # pre-zero the odd-j gap regions of both expT buffers:
            # (p, j=2jp+1, c in [256jp, 256jp+128)) is read by paired BMM2
            # but never written by exp.
            expT_bufs = []
            for i in range(2):
                e = expp.tile([P, SC, S], BF16, tag="expT", name=f"expT{i}")
                base = e[:]
                gap = bass.AP(tensor=base.tensor, offset=base.offset + S,
                              ap=[base.ap[0], [2 * S + 256, 4], [1, P]])
                nc.vector.memset(gap, 0.0)
                expT_bufs.append(e)

            pipe_qk = {}
            pipe_exp = {}
            pipe_cd = {}

            def emit_qk0():
                emit_qk_half(0, 0)
                emit_qk_half(0, 1)

            def emit_qk_half(h, hs):
                if hs == 0:
                    qt = qkp.tile([64, S], BF16, tag="qT", name=f"qT{h}")
                    kt = qkp.tile([64, S], BF16, tag="kT", name=f"kT{h}")
                    pipe_qk[h] = (qt, kt)
                qt, kt = pipe_qk[h]
                pq = psQK.tile([P, 512], F32, tag="pq", name="pq")
                pk = psQK.tile([P, 512], F32, tag="pk", name="pk")
                for s2 in range(2):
                    scol = 512 * hs + 256 * s2
                    for t in range(4):
                        nc.tensor.matmul(
                            pq[0:64, 256 * s2:256 * s2 + 256],
                            wq[:, 2 * t:2 * t + 2, 64 * h:64 * h + 64],
                            xT[:, 2 * t:2 * t + 2, scol:scol + 256],
                            start=(t == 0), stop=(t == 3),
                            perf_mode=DR, skip_group_check=True)
                    for t in range(4):
                        nc.tensor.matmul(
                            pk[0:64, 256 * s2:256 * s2 + 256],
                            wk[:, 2 * t:2 * t + 2, 64 * h:64 * h + 64],
                            xT[:, 2 * t:2 * t + 2, scol:scol + 256],
                            start=(t == 0), stop=(t == 3),
                            perf_mode=DR, skip_group_check=True)
                cols = slice(512 * hs, 512 * (hs + 1))
                nc.vector.tensor_copy(qt[:, cols], pq[0:64, :])
                nc.vector.tensor_copy(kt[:, cols], pk[0:64, :])

            def emit_scores_js(h, js):
                if js[0] == 0:
                    pipe_exp[h] = expp.tile([P, SC, S], BF16, tag="expT",
                                            name=f"expT_h{h}")
                qt, kt = pipe_qk[h]
                expT = pipe_exp[h]
                for j in js:
                    lo = P * j
                    kslice = kt[:, lo:lo + P]
                    if lo < 512:
                        ps = psS.tile([P, 512], F32, tag="sc", name="sc")
                        nc.tensor.matmul(ps[:, lo:512], kslice,
                                         qt[:, lo:512],
                                         start=True, stop=True,
                                         skip_group_check=True)
                        nc.scalar.activation(expT[:, j, lo:512], ps[:, lo:512],
                                             AF.Exp, bias=0.0, scale=SCALE)
                        ps2 = psS.tile([P, 512], F32, tag="sc", name="sc2")
                        nc.tensor.matmul(ps2[:], kslice, qt[:, 512:S],
                                         start=True, stop=True,
                                         skip_group_check=True)
                        nc.scalar.activation(expT[:, j, 512:S], ps2[:],
                                             AF.Exp, bias=0.0, scale=SCALE)
                    else:
                        ps = psS.tile([P, 512], F32, tag="sc", name="sc")
                        nc.tensor.matmul(ps[:, lo - 512:512], kslice,
                                         qt[:, lo:S],
                                         start=True, stop=True,
                                         skip_group_check=True)
                        nc.scalar.activation(expT[:, j, lo:S],
                                             ps[:, lo - 512:512],
                                             AF.Exp, bias=0.0, scale=SCALE)
                if js[-1] == SC - 1:
                    # strict-upper mask on the 8 diagonal blocks, one Pool op
                    del pipe_qk[h]
                    base = expT[:]
                    diag = bass.AP(tensor=base.tensor, offset=base.offset,
                                   ap=[base.ap[0], [S + P, SC], [1, P]])
                    mb = mask_rep[:]
                    mask3 = bass.AP(tensor=mb.tensor, offset=mb.offset,
                                    ap=[mb.ap[0], [P, SC], [1, P]])
                    nc.vector.tensor_mul(diag, diag, mask3)

            def emit_bmm2_mm(h):
                expT = pipe_exp.pop(h)
                cds = []
                for n in range(2):
                    cd = psCD.tile([P, 512], F32, tag=f"cd{n}", name=f"cd{n}")
                    cds.append(cd)
                    jmax = (512 * n + 511) // P
                    for j in range(jmax + 1):
                        c0 = max(512 * n, P * j)
                        nc.tensor.matmul(
                            cd[0:DK + 1, c0 - 512 * n:512],
                            vaug[:, j, h, :],
                            expT[:, j, c0:512 * (n + 1)],
                            start=(j == 0), stop=(j == jmax),
                            skip_group_check=True)
                    rt = recp.tile([P, 512], F32R, tag="rec", name="rec")
                    nc.vector.tensor_copy(rt[DK:DK + 1, :], cd[DK:DK + 1, :])
                    cds.append(rt)
                pipe_cd[h] = cds

            def emit_bmm2_norm(h):
                cd0, rt0, cd1, rt1 = pipe_cd.pop(h)
                ch, off = h // 2, 64 * (h % 2)
                tmp = None
                if off:
                    tmp = recp.tile([64, S], BF16, tag="tmp", name="tmp")
                for n, (cd, rt) in enumerate(((cd0, rt0), (cd1, rt1))):
                    # broadcast den across 64 partitions (PE), reciprocal to
                    # SBUF, normalize (single-PSUM-input multiply), then
                    # convert f32 -> bf16 on alternating engines.
                    pb = psS.tile([P, 512], F32, tag="sc", name="bc")
                    nc.tensor.matmul(pb[0:64, 0:512],
                                     ones1r[DK:DK + 1, :],
                                     rt[DK:DK + 1, :],
                                     start=True, stop=True,
                                     skip_group_check=True)
                    rb = recp.tile([64, 512], F32, tag="rb", name="rb")
                    nc.vector.reciprocal(rb[:], pb[0:64, 0:512])
                    dst = (concatT[0:64, ch, 512 * n:512 * (n + 1)]
                           if off == 0 else tmp[:, 512 * n:512 * (n + 1)])
                    nc.vector.tensor_mul(dst, cd[0:DK, :], rb[:])
                if off:
                    nc.sync.dma_start(concatT[64:P, ch, :], tmp[:])

            emit_qk0()

            # ================= phase V: V projection =================
            for m in range(SC):
                for nf in range(2):
                    ps = psS.tile([P, 512], F32, tag="sc", name="v")[:, :]
                    for t in range(DC):
                        nc.tensor.matmul(
                            ps, xTb[:, t, P * m:P * (m + 1)],
                            wv[:, t, 512 * nf:512 * (nf + 1)],
                            start=(t == 0), stop=(t == DC - 1),
                            skip_group_check=True)
                    nc.scalar.activation(
                        vaug[:, m, 8 * nf:8 * (nf + 1), 0:DK],
                        ps.rearrange("p (h d) -> p h d", d=DK),
                        AF.Identity)

            for h in range(1, H + 2):
                if h <= H:
                    emit_scores_js(h - 1, [0, 1])
                if h < H:
                    emit_qk_half(h, 0)
                if h >= 2:
                    emit_bmm2_mm(h - 2)
                if h <= H:
                    emit_scores_js(h - 1, [2, 3])
                if h < H:
                    emit_qk_half(h, 1)
                if h >= 2:
                    emit_bmm2_norm(h - 2)
                if h <= H:
                    emit_scores_js(h - 1, [4, 5, 6, 7])

        attp_cm.__exit__(None, None, None)

        # ============ phase C/D: out-proj + residual + LN1 + transpose ======
        with tc.tile_pool(name="xs", bufs=4) as xs, \
             tc.tile_pool(name="psC", bufs=4, space="PSUM") as psC, \
             tc.tile_pool(name="psT", bufs=3, space="PSUM") as psT:
            def emit_transpose(m):
                for cq in range(2):
                    pt = psT.tile([P, 4, P], F32, tag="t", name="t")
                    for ci in range(4):
                        c = 4 * cq + ci
                        nc.tensor.matmul(
                            pt[:, ci, :],
                            h1[:, m, P * c:P * (c + 1)],
                            ident[:], is_transpose=True,
                            start=True, stop=True, skip_group_check=True)
                    nc.scalar.activation(
                        h1T[:, 4 * cq:4 * (cq + 1), P * m:P * (m + 1)],
                        pt[:], AF.Identity)

            for m in range(SC + 1):
                if m < SC:
                    xm = xs.tile([P, D], F32, tag="x", name=f"x{m}")
                    nc.sync.dma_start(xm[:], x_d[P * m:P * (m + 1), :])
                    stm = sm.tile([P, 2, 6], F32, tag="st", name="st")
                    for n in range(2):
                        pc = psC.tile([P, 512], F32, tag="c", name="c")
                        for c in range(DC):
                            nc.tensor.matmul(
                                pc[:], concatT[:, c, P * m:P * (m + 1)],
                                wo[:, c, 512 * n:512 * (n + 1)],
                                start=(c == 0), stop=(c == DC - 1),
                                skip_group_check=True)
                        cols = slice(512 * n, 512 * (n + 1))
                        nc.vector.tensor_add(h1[:, m, cols], pc[:], xm[:, cols])
                        nc.vector.bn_stats(stm[:, n, :], h1[:, m, cols])
                    ln_finish(stm, h1[:, m, :])
                if m >= 1:
                    emit_transpose(m - 1)

        # ================= phase E: FFN1 =================
        ftp_cm = tc.tile_pool(name="ftp", bufs=1)
        ftp = ftp_cm.__enter__()
        fT = ftp.tile([P, FC, S], BF16)
        with tc.tile_pool(name="w1s", bufs=6) as w1s, \
             tc.tile_pool(name="psF", bufs=2, space="PSUM") as psF:
            for c in range(FC):
                wt = w1s.tile([P, DC, P], W1DT, tag="w1", name=f"w1_{c}")
                nc.sync.dma_start(wt[:], w1_d[c])
                pf = psF.tile([P, 4, 256], F32, tag="f", name="f")
                if FFN1_BF16:
                    for sh in range(2):
                        for t in range(DC):
                            nc.tensor.matmul(
                                pf[:, 2 * sh:2 * sh + 2, :].rearrange(
                                    "p a b -> p (a b)"),
                                wt[:, t, :], h1T[:, t, 512 * sh:512 * (sh + 1)],
                                start=(t == 0), stop=(t == DC - 1),
                                skip_group_check=True)
                else:
                    for t4 in range(4):
                        for a in range(2):
                            for kp in range(4):
                                nc.tensor.matmul(
                                    pf[64 * a:64 * a + 64, t4, :],
                                    wt[:, 2 * kp:2 * kp + 2, 64 * a:64 * a + 64],
                                    h1T[:, 2 * kp:2 * kp + 2,
                                        256 * t4:256 * (t4 + 1)],
                                    start=(kp == 0), stop=(kp == 3),
                                    perf_mode=DR, skip_group_check=True)
                nc.scalar.activation(fT[:, c, :],
                                     pf[:].rearrange("p a b -> p (a b)"),
                                     AF.Relu)

        # ================= phase F: FFN2 + LN2 + out =================
        with tc.tile_pool(name="w2s", bufs=4) as w2s, \
             tc.tile_pool(name="psG", bufs=1, space="PSUM") as psG:
            wts = {}

            def w2dma(i):
                half, c = divmod(i, FC)
                wt = w2s.tile([P, D], BF16, tag="w2", name=f"w2_{half}_{c}")
                nc.sync.dma_start(wt[:], w2_d[P * c:P * (c + 1), :])
                wts[i] = wt

            w2dma(0)
            w2dma(1)
            for half in range(2):
                gt = [[psG.tile([P, 512], F32, tag=f"g{mi}{n}",
                                name=f"g{half}{mi}{n}")
                       for n in range(2)] for mi in range(4)]
                for c in range(FC):
                    i = half * FC + c
                    if i + 2 < 2 * FC:
                        w2dma(i + 2)
                    wt2 = wts.pop(i)
                    for mi in range(4):
                        m = 4 * half + mi
                        for n in range(2):
                            nc.tensor.matmul(
                                gt[mi][n][:], fT[:, c, P * m:P * (m + 1)],
                                wt2[:, 512 * n:512 * (n + 1)],
                                start=(c == 0), stop=(c == FC - 1),
                                skip_group_check=True)
                for mi in range(4):
                    m = 4 * half + mi
                    stm = sm.tile([P, 2, 6], F32, tag="st", name="st2")
                    for n in range(2):
                        cols = slice(512 * n, 512 * (n + 1))
                        nc.vector.tensor_add(h1[:, m, cols], gt[mi][n][:],
                                             h1[:, m, cols])
                        nc.vector.bn_stats(stm[:, n, :], h1[:, m, cols])
                    ln_finish(stm, h1[:, m, :])
                    nc.sync.dma_start(out_d[P * m:P * (m + 1), :], h1[:, m, :])
        ftp_cm.__exit__(None, None, None)

    nc.compile()
    return nc


_cached = None


def _get_prog():
    global _cached
    if _cached is None:
        _cached = _build()
    return _cached


def _ln_np(v, g, b):
    mu = v.mean(-1, keepdims=True)
    var = v.var(-1, keepdims=True)
    return (v - mu) / np.sqrt(var + EPS) * g + b


def _host_row0(inputs):
    """Exact f32 recompute of output row 0 (attention row 0 is zero-padded)."""
    x0 = np.asarray(inputs["x"], np.float32)[:, 0, :]          # [B, D]
    h1 = _ln_np(x0 + inputs["bo"][None, :], inputs["g1"], inputs["beta1"])
    f1 = np.maximum(h1 @ inputs["W1"] + inputs["b1"], 0.0)
    ffn = f1 @ inputs["W2"] + inputs["b2"]
    return _ln_np(h1 + ffn, inputs["g3"], inputs["beta3"])


def _fallback_np(inputs):
    x = np.asarray(inputs["x"], np.float32)
    q = x @ inputs["Wq"] + inputs["bq"]
    k = x @ inputs["Wk"] + inputs["bk"]
    v = x @ inputs["Wv"] + inputs["bv"]

    def heads(t):
        return t.reshape(B, S, H, DK).transpose(0, 2, 1, 3)
    q, k, v = heads(q), heads(k), heads(v)
    sc = np.einsum('bhqd,bhkd->bhqk', q, k).astype(np.float32) / 8.0
    i = np.arange(S)[:, None]
    j = np.arange(S)[None, :]
    sc = np.where((j < i)[None, None], sc, -1e32)
    sc -= sc.max(-1, keepdims=True)
    e = np.exp(sc)
    att = e / e.sum(-1, keepdims=True)
    att = att * (i > 0)[None, None]
    ctx = np.einsum('bhqk,bhkd->bhqd', att, v)
    concat = ctx.transpose(0, 2, 1, 3).reshape(B, S, D)
    h1 = _ln_np(x + concat @ inputs["Wo"] + inputs["bo"],
                inputs["g1"], inputs["beta1"])
    f1 = np.maximum(h1 @ inputs["W1"] + inputs["b1"], 0.0)
    out = _ln_np(h1 + f1 @ inputs["W2"] + inputs["b2"],
                 inputs["g3"], inputs["beta3"])
    return out.astype(np.float32)


def _fast_path_ok(inputs):
    z = lambda n: not np.any(np.asarray(inputs[n]))
    o = lambda n: np.all(np.asarray(inputs[n]) == 1.0)
    return (z("bq") and z("bk") and z("bv") and z("bo") and z("b1")
            and z("b2") and z("beta1") and z("beta3") and o("g1") and o("g3"))


def kernel(**inputs):
    x = np.asarray(inputs["x"], dtype=np.float32)
    assert x.shape == (B, S, D)
    if not _fast_path_ok(inputs):
        return _fallback_np(inputs)

    f8 = lambda a: np.ascontiguousarray(np.asarray(a)).astype(NP_F8)
    bf = lambda a: np.ascontiguousarray(np.asarray(a)).astype(NP_BF16)
    w1 = np.asarray(inputs["W1"], np.float32)
    w1p = np.ascontiguousarray(
        w1.reshape(DC, P, FC, P).transpose(2, 1, 0, 3)).astype(
            NP_BF16 if FFN1_BF16 else NP_F8)
    common = dict(
        wq8=f8(inputs["Wq"]), wk8=f8(inputs["Wk"]), wvb=bf(inputs["Wv"]),
        wob=bf(inputs["Wo"]), w1p=w1p, w2b=bf(inputs["W2"]),
        mask8=np.triu(np.ones((P, P), np.float32), k=1).astype(NP_BF16),
        ident=np.eye(P, dtype=np.float32),
        onesf=np.ones((P, 64), np.float32),
    )
    in_maps = []
    for i in range(B):
        xi = np.ascontiguousarray(x[i])
        xiT = np.ascontiguousarray(xi.T)
        in_maps.append(dict(common, x32=xi, xT8=xiT.astype(NP_F8),
                            xTb=xiT.astype(NP_BF16)))
    nc = _get_prog()
    res = run_bass_kernel_spmd(nc, in_maps, list(range(B)))
    out = np.stack([res.results[i]["out"] for i in range(B)], axis=0)
    out[:, 0, :] = _host_row0(inputs)
    return out



# revision 3
# speedup vs baseline: 1.1308x; 1.1308x over previous
"""Trainium2 Bass kernel for nn_BasicBlock (dense transformer block).

Data-parallel over batch: B=8 batch elements, one per NeuronCore, no
collectives. fp8e4m3-DoubleRow on every GEMM that tolerates it, with
scaled hi/lo error-correction splits where single fp8 would break the
2e-2 budget.

Numerics scheme (validated in numpy prototype, proto3.py):
  - A GEMM a@W in "sbs" (split-both-scaled) form runs three fp8-DR GEMMs
    accumulating into one PSUM: a_hi@W_hi + a_lo@W_hi + (a_hi/8)@(8*W_lo).
    The x8/ :8 scaling keeps the W-residual above e4m3's subnormal floor
    (W ~ 0.02 scale would otherwise flush to zero). Residual error ~0.1%.
  - FFN1, FFN2, V-proj: sbs (exact-ish). Out-proj: sws (W-corrected,
    concat single-quant). Scores Q/K bf16 storage (stage A), BMM2 in fp8
    DR with paired key-chunks (K=256/instruction).
  - Predicted rel err ~ 8e-3 vs the 2e-2 budget.

Per-core layout (S=1024, D=1024, H=16, d_k=64, d_ff=4096):
  - Q/K projections fp8 DoubleRow as before; scoresT bf16 per
    (head, key-chunk), causal columns only; exp on ScalarE writes fp8
    expT; strict lower-triangle mask via diagonal-strided DVE multiply;
    BMM2 runs DR over key-chunk PAIRS (the pre-zeroed odd-j gap regions
    make the paired reads causal-safe); ones-augmented V accumulates the
    softmax denominator; normalize fused into the fp8 quantize multiply.
  - Attention row 0 (zero-pad row) produces NaNs on device; host
    recomputes output row 0 exactly in numpy.
  - Fast path assumes the reference's structural constants (biases 0,
    gammas 1, betas 0); other inputs fall back to numpy.
"""

import numpy as np
import ml_dtypes
import concourse.bass as bass
import concourse.tile as tile
from concourse import bacc, mybir
from concourse.bass_utils import run_bass_kernel_spmd

F32 = mybir.dt.float32
F32R = mybir.dt.float32r
F8 = mybir.dt.float8e4
BF16 = mybir.dt.bfloat16
AF = mybir.ActivationFunctionType
OP = mybir.AluOpType
DR = mybir.MatmulPerfMode.DoubleRow

NP_F8 = ml_dtypes.float8_e4m3
NP_BF16 = ml_dtypes.bfloat16

B, S, D, H, DK, DFF = 8, 1024, 1024, 16, 64, 4096
P = 128
DC = D // P       # 8 chunks of d_model
DP = DC // 2      # 4 K-pairs of d_model (DR)
FC = DFF // P     # 32 chunks of d_ff
FP2 = FC // 2     # 16 K-pairs of d_ff (DR)
SC = S // P       # 8 chunks of sequence
EPS = 1e-5
SCALE = 0.125     # 1/sqrt(DK)


def _build():
    nc = bacc.Bacc("TRN2", target_bir_lowering=False, debug=False, num_devices=B)

    # per-batch inputs
    xT_d = nc.dram_tensor("xT8", [D, S], F8, kind="ExternalInput").ap()
    xTl_d = nc.dram_tensor("xTl8", [D, S], F8, kind="ExternalInput").ap()
    xTh8_d = nc.dram_tensor("xTh8", [D, S], F8, kind="ExternalInput").ap()
    x_d = nc.dram_tensor("x32", [S, D], F32, kind="ExternalInput").ap()
    # common weights
    wq_d = nc.dram_tensor("wq8", [D, D], F8, kind="ExternalInput").ap()
    wk_d = nc.dram_tensor("wk8", [D, D], F8, kind="ExternalInput").ap()
    wvh_d = nc.dram_tensor("wvh", [D, D], F8, kind="ExternalInput").ap()
    wvl_d = nc.dram_tensor("wvl", [D, D], F8, kind="ExternalInput").ap()
    woh_d = nc.dram_tensor("woh", [D, D], F8, kind="ExternalInput").ap()
    wol_d = nc.dram_tensor("wol", [D, D], F8, kind="ExternalInput").ap()
    # host pre-arranged: w1p[c, p, t, f] = W1[128t+p, 128c+f]
    w1h_d = nc.dram_tensor("w1h", [FC, P, DC, P], F8, kind="ExternalInput").ap()
    w1l_d = nc.dram_tensor("w1l", [FC, P, DC, P], F8, kind="ExternalInput").ap()
    w2h_d = nc.dram_tensor("w2h", [DFF, D], F8, kind="ExternalInput").ap()
    w2l_d = nc.dram_tensor("w2l", [DFF, D], F8, kind="ExternalInput").ap()
    mask_d = nc.dram_tensor("mask8", [P, P], F8, kind="ExternalInput").ap()
    id_d = nc.dram_tensor("ident", [P, P], F32, kind="ExternalInput").ap()
    ones_d = nc.dram_tensor("onesf", [P, 64], F32, kind="ExternalInput").ap()
    out_d = nc.dram_tensor("out", [S, D], F32, kind="ExternalOutput").ap()

    with tile.TileContext(nc) as tc:
      with tc.tile_pool(name="singles", bufs=1) as sg, \
           tc.tile_pool(name="sm", bufs=6) as sm:
        mask_rep = sg.tile([P, SC, P], F8)
        ident = sg.tile([P, P], F32)
        ones1r = sg.tile([P, 64], F32R)
        eps_t = sg.tile([P, 1], F32)
        h1 = sg.tile([P, SC, D], F32)
        h1T_hi = sg.tile([P, DC, S], F8)
        h1T_lo = sg.tile([P, DC, S], F8)
        h1T_h8 = sg.tile([P, DC, S], F8)

        nc.vector.memset(eps_t[:], EPS)
        nc.sync.dma_start(ident[:], id_d)
        nc.sync.dma_start(ones1r[:], ones_d.bitcast(F32R))
        # mask replicated 8x along a middle dim for the diagonal-AP multiply
        mask_bc = bass.AP(tensor=mask_d.tensor, offset=mask_d.offset,
                          ap=[[P, P], [0, SC], [1, P]])
        nc.sync.dma_start(mask_rep[:], mask_bc)

        # pool spanning V-proj .. out-proj/LN1
        midp_cm = tc.tile_pool(name="midp", bufs=1)
        mp_ = midp_cm.__enter__()
        concatT = mp_.tile([P, DC, S], F8)
        concat8 = mp_.tile([P, DC, S], F8)
        woh = mp_.tile([P, DC, D], F8)
        wol = mp_.tile([P, DC, D], F8)

        attp_cm = tc.tile_pool(name="attp", bufs=1)
        ap_ = attp_cm.__enter__()
        xT = ap_.tile([P, DC, S], F8)
        xTl = ap_.tile([P, DC, S], F8)
        xTh8 = ap_.tile([P, DC, S], F8)
        wq = ap_.tile([P, DC, D], F8)
        wk = ap_.tile([P, DC, D], F8)
        wvh = ap_.tile([P, DC, D], F8)
        wvl = ap_.tile([P, DC, D], F8)
        vaug = ap_.tile([P, SC, H, DK + 1], F8)

        # x / V-weight loads split so the first V matmuls can start early
        xT_r = xT_d.rearrange("(c p) s -> p c s", p=P)
        wvh_r = wvh_d.rearrange("(c p) d -> p c d", p=P)
        nc.sync.dma_start(xT[:, 0:4, :], xT_r[:, 0:4, :])
        nc.sync.dma_start(wvh[:, 0:4, :], wvh_r[:, 0:4, :])
        nc.sync.dma_start(xT[:, 4:8, :], xT_r[:, 4:8, :])
        nc.sync.dma_start(wvh[:, 4:8, :], wvh_r[:, 4:8, :])
        nc.sync.dma_start(xTl[:], xTl_d.rearrange("(c p) s -> p c s", p=P))
        nc.sync.dma_start(xTh8[:], xTh8_d.rearrange("(c p) s -> p c s", p=P))
        nc.sync.dma_start(wvl[:], wvl_d.rearrange("(c p) d -> p c d", p=P))
        nc.sync.dma_start(wq[:], wq_d.rearrange("(c p) d -> p c d", p=P))
        nc.sync.dma_start(wk[:], wk_d.rearrange("(c p) d -> p c d", p=P))
        nc.sync.dma_start(woh[:], woh_d.rearrange("(c p) d -> p c d", p=P))
        nc.sync.dma_start(wol[:], wol_d.rearrange("(c p) d -> p c d", p=P))
        nc.vector.memset(vaug[:, :, :, DK:DK + 1], 1.0)

        def ln_finish(stm, ap_full):
            # stm [P, 2, 6] bn_stats pairs; ap_full [P, 1024] in SBUF
            mv = sm.tile([P, 2], F32, tag="mv", name="mv")
            rs = sm.tile([P, 1], F32, tag="rs", name="rs")
            nb = sm.tile([P, 1], F32, tag="nb", name="nb")
            nc.vector.bn_aggr(mv[:], stm[:])
            nc.scalar.activation(rs[:], mv[:, 1:2], AF.Sqrt,
                                 bias=eps_t[:], scale=1.0)
            nc.vector.reciprocal(rs[:], rs[:])
            nc.vector.tensor_scalar(nb[:], mv[:, 0:1], rs[:], -1.0,
                                    op0=OP.mult, op1=OP.mult)
            nc.scalar.activation(ap_full, ap_full, AF.Identity,
                                 bias=nb[:], scale=rs[:])

        # ================= attention =================
        with tc.tile_pool(name="qkp", bufs=3) as qkp, \
             tc.tile_pool(name="expp", bufs=2) as expp, \
             tc.tile_pool(name="recp", bufs=1) as recp, \
             tc.tile_pool(name="psQK", bufs=1, space="PSUM") as psQK, \
             tc.tile_pool(name="psS", bufs=4, space="PSUM") as psS, \
             tc.tile_pool(name="psCD", bufs=1, space="PSUM") as psCD:

            # pre-zero the odd-j gap regions of both expT buffers:
            # (p, j=2jp+1, c in [256jp, 256jp+128)) is read by paired BMM2
            # but never written by exp.
            for i in range(2):
                e = expp.tile([P, SC, S], F8, tag="expT", name=f"expT{i}")
                base = e[:]
                gap = bass.AP(tensor=base.tensor, offset=base.offset + S,
                              ap=[base.ap[0], [2 * S + 256, 4], [1, P]])
                nc.vector.memset(gap, 0.0)

            pipe_qk = {}
            pipe_exp = {}
            pipe_cd = {}

            def emit_qk0():
                emit_qk_half(0, 0)
                emit_qk_half(0, 1)

            def emit_qk_half(h, hs):
                if hs == 0:
                    qt = qkp.tile([64, S], BF16, tag="qT", name=f"qT{h}")
                    kt = qkp.tile([64, S], BF16, tag="kT", name=f"kT{h}")
                    pipe_qk[h] = (qt, kt)
                qt, kt = pipe_qk[h]
                pq = psQK.tile([P, 512], F32, tag="pq", name="pq")
                pk = psQK.tile([P, 512], F32, tag="pk", name="pk")
                for s2 in range(2):
                    scol = 512 * hs + 256 * s2
                    for t in range(4):
                        nc.tensor.matmul(
                            pq[0:64, 256 * s2:256 * s2 + 256],
                            wq[:, 2 * t:2 * t + 2, 64 * h:64 * h + 64],
                            xT[:, 2 * t:2 * t + 2, scol:scol + 256],
                            start=(t == 0), stop=(t == 3),
                            perf_mode=DR, skip_group_check=True)
                    for t in range(4):
                        nc.tensor.matmul(
                            pk[0:64, 256 * s2:256 * s2 + 256],
                            wk[:, 2 * t:2 * t + 2, 64 * h:64 * h + 64],
                            xT[:, 2 * t:2 * t + 2, scol:scol + 256],
                            start=(t == 0), stop=(t == 3),
                            perf_mode=DR, skip_group_check=True)
                cols = slice(512 * hs, 512 * (hs + 1))
                nc.vector.tensor_copy(qt[:, cols], pq[0:64, :])
                nc.vector.tensor_copy(kt[:, cols], pk[0:64, :])

            def emit_scores_js(h, js):
                if js[0] == 0:
                    pipe_exp[h] = expp.tile([P, SC, S], F8, tag="expT",
                                            name=f"expT_h{h}")
                qt, kt = pipe_qk[h]
                expT = pipe_exp[h]
                for j in js:
                    lo = P * j
                    kslice = kt[:, lo:lo + P]
                    if lo < 512:
                        ps = psS.tile([P, 512], F32, tag="sc", name="sc")
                        nc.tensor.matmul(ps[:, lo:512], kslice,
                                         qt[:, lo:512],
                                         start=True, stop=True,
                                         skip_group_check=True)
                        nc.scalar.activation(expT[:, j, lo:512], ps[:, lo:512],
                                             AF.Exp, bias=0.0, scale=SCALE)
                        ps2 = psS.tile([P, 512], F32, tag="sc", name="sc2")
                        nc.tensor.matmul(ps2[:], kslice, qt[:, 512:S],
                                         start=True, stop=True,
                                         skip_group_check=True)
                        nc.scalar.activation(expT[:, j, 512:S], ps2[:],
                                             AF.Exp, bias=0.0, scale=SCALE)
                    else:
                        ps = psS.tile([P, 512], F32, tag="sc", name="sc")
                        nc.tensor.matmul(ps[:, lo - 512:512], kslice,
                                         qt[:, lo:S],
                                         start=True, stop=True,
                                         skip_group_check=True)
                        nc.scalar.activation(expT[:, j, lo:S],
                                             ps[:, lo - 512:512],
                                             AF.Exp, bias=0.0, scale=SCALE)
                if js[-1] == SC - 1:
                    # strict-upper mask on the 8 diagonal blocks, one DVE op
                    del pipe_qk[h]
                    base = expT[:]
                    diag = bass.AP(tensor=base.tensor, offset=base.offset,
                                   ap=[base.ap[0], [S + P, SC], [1, P]])
                    mb = mask_rep[:]
                    mask3 = bass.AP(tensor=mb.tensor, offset=mb.offset,
                                    ap=[mb.ap[0], [P, SC], [1, P]])
                    nc.vector.tensor_mul(diag, diag, mask3)

            # BMM2: fp8 DoubleRow over key-chunk pairs (K=256/instruction)
            def emit_bmm2_mm(h):
                expT = pipe_exp.pop(h)
                cds = []
                for n in range(2):
                    cd = psCD.tile([P, 512], F32, tag=f"cd{n}", name=f"cd{n}")
                    cds.append(cd)
                    # causal pairs for this column half
                    if n == 0:
                        pairs = [(0, 0), (2, 256)]
                    else:
                        pairs = [(0, 512), (2, 512), (4, 512), (6, 768)]
                    for pi, (j, c0) in enumerate(pairs):
                        nc.tensor.matmul(
                            cd[0:DK + 1, c0 - 512 * n:512],
                            vaug[:, j:j + 2, h, :],
                            expT[:, j:j + 2, c0:512 * (n + 1)],
                            start=(pi == 0), stop=(pi == len(pairs) - 1),
                            perf_mode=DR, skip_group_check=True)
                    rt = recp.tile([P, 512], F32R, tag="rec", name="rec")
                    nc.vector.tensor_copy(rt[DK:DK + 1, :], cd[DK:DK + 1, :])
                    cds.append(rt)
                pipe_cd[h] = cds

            def emit_bmm2_norm(h):
                cd0, rt0, cd1, rt1 = pipe_cd.pop(h)
                ch, off = h // 2, 64 * (h % 2)
                tmp = None
                if off:
                    tmp = recp.tile([64, S], F8, tag="tmp", name="tmp")
                for n, (cd, rt) in enumerate(((cd0, rt0), (cd1, rt1))):
                    # broadcast den across 64 partitions (PE), reciprocal to
                    # SBUF, normalize (single-PSUM-input multiply) with fp8
                    # quantize fused in.
                    pb = psS.tile([P, 512], F32, tag="sc", name="bc")
                    nc.tensor.matmul(pb[0:64, 0:512],
                                     ones1r[DK:DK + 1, :],
                                     rt[DK:DK + 1, :],
                                     start=True, stop=True,
                                     skip_group_check=True)
                    rb = recp.tile([64, 512], F32, tag="rb", name="rb")
                    nc.vector.reciprocal(rb[:], pb[0:64, 0:512])
                    dst = (concatT[0:64, ch, 512 * n:512 * (n + 1)]
                           if off == 0 else tmp[:, 512 * n:512 * (n + 1)])
                    nc.vector.tensor_mul(dst, cd[0:DK, :], rb[:])
                if off:
                    nc.sync.dma_start(concatT[64:P, ch, :], tmp[:])
                    # concat/8 for the Wo-residual correction GEMM (sws)
                    nc.gpsimd.tensor_scalar_mul(concat8[:, ch, :],
                                                concatT[:, ch, :], 0.125)

            emit_qk0()

            # ================= phase V: V projection (sbs, 3 DR GEMMs) ======
            for m in range(SC):
                for nf in range(2):
                    ps = psS.tile([P, 512], F32, tag="sc", name="v")[:, :]
                    ncol = slice(512 * nf, 512 * (nf + 1))
                    mcol = slice(P * m, P * (m + 1))
                    for t in range(DP):
                        tp = slice(2 * t, 2 * t + 2)
                        nc.tensor.matmul(
                            ps, xT[:, tp, mcol], wvh[:, tp, ncol],
                            start=(t == 0), stop=False,
                            perf_mode=DR, skip_group_check=True)
                    for t in range(DP):
                        tp = slice(2 * t, 2 * t + 2)
                        nc.tensor.matmul(
                            ps, xTl[:, tp, mcol], wvh[:, tp, ncol],
                            start=False, stop=False,
                            perf_mode=DR, skip_group_check=True)
                    for t in range(DP):
                        tp = slice(2 * t, 2 * t + 2)
                        nc.tensor.matmul(
                            ps, xTh8[:, tp, mcol], wvl[:, tp, ncol],
                            start=False, stop=(t == DP - 1),
                            perf_mode=DR, skip_group_check=True)
                    nc.scalar.activation(
                        vaug[:, m, 8 * nf:8 * (nf + 1), 0:DK],
                        ps.rearrange("p (h d) -> p h d", d=DK),
                        AF.Identity)

            for h in range(1, H + 2):
                if h <= H:
                    emit_scores_js(h - 1, [0, 1])
                if h < H:
                    emit_qk_half(h, 0)
                if h >= 2:
                    emit_bmm2_mm(h - 2)
                if h <= H:
                    emit_scores_js(h - 1, [2, 3])
                if h < H:
                    emit_qk_half(h, 1)
                if h >= 2:
                    emit_bmm2_norm(h - 2)
                if h <= H:
                    emit_scores_js(h - 1, [4, 5, 6, 7])

        attp_cm.__exit__(None, None, None)

        # ===== phase C/D: out-proj (sws, 2 DR GEMMs) + residual + LN1 + T ====
        with tc.tile_pool(name="xs", bufs=4) as xs, \
             tc.tile_pool(name="psC", bufs=4, space="PSUM") as psC, \
             tc.tile_pool(name="psT", bufs=3, space="PSUM") as psT:
            def emit_transpose(m):
                for cq in range(2):
                    pt = psT.tile([P, 4, P], F32, tag="t", name="t")
                    for ci in range(4):
                        c = 4 * cq + ci
                        nc.tensor.matmul(
                            pt[:, ci, :],
                            h1[:, m, P * c:P * (c + 1)],
                            ident[:], is_transpose=True,
                            start=True, stop=True, skip_group_check=True)
                    cols = slice(P * m, P * (m + 1))
                    cq4 = slice(4 * cq, 4 * (cq + 1))
                    hi = h1T_hi[:, cq4, cols]
                    nc.scalar.activation(hi, pt[:], AF.Identity)
                    nc.vector.scalar_tensor_tensor(
                        h1T_lo[:, cq4, cols], pt[:], 1.0, hi,
                        op0=OP.mult, op1=OP.subtract)
                    nc.gpsimd.tensor_scalar_mul(
                        h1T_h8[:, cq4, cols], hi, 0.125)

            for m in range(SC + 1):
                if m < SC:
                    xm = xs.tile([P, D], F32, tag="x", name=f"x{m}")
                    nc.sync.dma_start(xm[:], x_d[P * m:P * (m + 1), :])
                    stm = sm.tile([P, 2, 6], F32, tag="st", name="st")
                    mcol = slice(P * m, P * (m + 1))
                    for n in range(2):
                        pc = psC.tile([P, 512], F32, tag="c", name="c")
                        ncol = slice(512 * n, 512 * (n + 1))
                        for c in range(DP):
                            cp = slice(2 * c, 2 * c + 2)
                            nc.tensor.matmul(
                                pc[:], concatT[:, cp, mcol], woh[:, cp, ncol],
                                start=(c == 0), stop=False,
                                perf_mode=DR, skip_group_check=True)
                        for c in range(DP):
                            cp = slice(2 * c, 2 * c + 2)
                            nc.tensor.matmul(
                                pc[:], concat8[:, cp, mcol], wol[:, cp, ncol],
                                start=False, stop=(c == DP - 1),
                                perf_mode=DR, skip_group_check=True)
                        nc.vector.tensor_add(h1[:, m, ncol], pc[:], xm[:, ncol])
                        nc.vector.bn_stats(stm[:, n, :], h1[:, m, ncol])
                    ln_finish(stm, h1[:, m, :])
                if m >= 1:
                    emit_transpose(m - 1)

        midp_cm.__exit__(None, None, None)

        # ================= phase E: FFN1 (sbs, 3 DR GEMMs) =================
        ftp_cm = tc.tile_pool(name="ftp", bufs=1)
        ftp = ftp_cm.__enter__()
        fT_hi = ftp.tile([P, FC, S], F8)
        fT_lo = ftp.tile([P, FC, S], F8)
        fT_h8 = ftp.tile([P, FC, S], F8)
        with tc.tile_pool(name="w1s", bufs=6) as w1s, \
             tc.tile_pool(name="psF", bufs=2, space="PSUM") as psF:
            for c in range(FC):
                wth = w1s.tile([P, DC, P], F8, tag="w1h", name=f"w1h_{c}")
                wtl = w1s.tile([P, DC, P], F8, tag="w1l", name=f"w1l_{c}")
                nc.sync.dma_start(wth[:], w1h_d[c])
                nc.sync.dma_start(wtl[:], w1l_d[c])
                pf = psF.tile([P, 2, 512], F32, tag="f", name="f")
                for sh in range(2):
                    cols = slice(512 * sh, 512 * (sh + 1))
                    for t in range(DP):
                        tp = slice(2 * t, 2 * t + 2)
                        nc.tensor.matmul(
                            pf[:, sh, :], wth[:, tp, :], h1T_hi[:, tp, cols],
                            start=(t == 0), stop=False,
                            perf_mode=DR, skip_group_check=True)
                    for t in range(DP):
                        tp = slice(2 * t, 2 * t + 2)
                        nc.tensor.matmul(
                            pf[:, sh, :], wth[:, tp, :], h1T_lo[:, tp, cols],
                            start=False, stop=False,
                            perf_mode=DR, skip_group_check=True)
                    for t in range(DP):
                        tp = slice(2 * t, 2 * t + 2)
                        nc.tensor.matmul(
                            pf[:, sh, :], wtl[:, tp, :], h1T_h8[:, tp, cols],
                            start=False, stop=(t == DP - 1),
                            perf_mode=DR, skip_group_check=True)
                pff = pf[:].rearrange("p a b -> p (a b)")
                nc.scalar.activation(fT_hi[:, c, :], pff, AF.Relu)
                nc.vector.scalar_tensor_tensor(
                    fT_lo[:, c, :], pff, 0.0, fT_hi[:, c, :],
                    op0=OP.max, op1=OP.subtract)
                nc.gpsimd.tensor_scalar_mul(fT_h8[:, c, :], fT_hi[:, c, :],
                                            0.125)

        # ============ phase F: FFN2 (sbs, 3 DR GEMMs) + LN2 + out ===========
        w2h_r = w2h_d.rearrange("(c p) d -> p c d", p=P)
        w2l_r = w2l_d.rearrange("(c p) d -> p c d", p=P)
        with tc.tile_pool(name="w2s", bufs=4) as w2s, \
             tc.tile_pool(name="psG", bufs=1, space="PSUM") as psG:
            wts = {}

            def w2dma(i):
                half, c2 = divmod(i, FP2)
                wh = w2s.tile([P, 2, D], F8, tag="w2h", name=f"w2h_{half}_{c2}")
                wl = w2s.tile([P, 2, D], F8, tag="w2l", name=f"w2l_{half}_{c2}")
                cp = slice(2 * c2, 2 * c2 + 2)
                nc.sync.dma_start(wh[:], w2h_r[:, cp, :])
                nc.sync.dma_start(wl[:], w2l_r[:, cp, :])
                wts[i] = (wh, wl)

            w2dma(0)
            w2dma(1)
            for half in range(2):
                gt = [[psG.tile([P, 512], F32, tag=f"g{mi}{n}",
                                name=f"g{half}{mi}{n}")
                       for n in range(2)] for mi in range(4)]
                for c2 in range(FP2):
                    i = half * FP2 + c2
                    if i + 2 < 2 * FP2:
                        w2dma(i + 2)
                    wh, wl = wts.pop(i)
                    cp = slice(2 * c2, 2 * c2 + 2)
                    for mi in range(4):
                        m = 4 * half + mi
                        mcol = slice(P * m, P * (m + 1))
                        for n in range(2):
                            ncol = slice(512 * n, 512 * (n + 1))
                            nc.tensor.matmul(
                                gt[mi][n][:], fT_hi[:, cp, mcol],
                                wh[:, :, ncol],
                                start=(c2 == 0), stop=False,
                                perf_mode=DR, skip_group_check=True)
                            nc.tensor.matmul(
                                gt[mi][n][:], fT_lo[:, cp, mcol],
                                wh[:, :, ncol],
                                start=False, stop=False,
                                perf_mode=DR, skip_group_check=True)
                            nc.tensor.matmul(
                                gt[mi][n][:], fT_h8[:, cp, mcol],
                                wl[:, :, ncol],
                                start=False, stop=(c2 == FP2 - 1),
                                perf_mode=DR, skip_group_check=True)
                for mi in range(4):
                    m = 4 * half + mi
                    stm = sm.tile([P, 2, 6], F32, tag="st", name="st2")
                    for n in range(2):
                        cols = slice(512 * n, 512 * (n + 1))
                        nc.vector.tensor_add(h1[:, m, cols], gt[mi][n][:],
                                             h1[:, m, cols])
                        nc.vector.bn_stats(stm[:, n, :], h1[:, m, cols])
                    ln_finish(stm, h1[:, m, :])
                    nc.sync.dma_start(out_d[P * m:P * (m + 1), :], h1[:, m, :])
        ftp_cm.__exit__(None, None, None)

    nc.compile()
    return nc


_cached = None


def _get_prog():
    global _cached
    if _cached is None:
        _cached = _build()
    return _cached


def _ln_np(v, g, b):
    mu = v.mean(-1, keepdims=True)
    var = v.var(-1, keepdims=True)
    return (v - mu) / np.sqrt(var + EPS) * g + b


def _host_row0(inputs):
    """Exact f32 recompute of output row 0 (attention row 0 is zero-padded)."""
    x0 = np.asarray(inputs["x"], np.float32)[:, 0, :]          # [B, D]
    h1 = _ln_np(x0 + inputs["bo"][None, :], inputs["g1"], inputs["beta1"])
    f1 = np.maximum(h1 @ inputs["W1"] + inputs["b1"], 0.0)
    ffn = f1 @ inputs["W2"] + inputs["b2"]
    return _ln_np(h1 + ffn, inputs["g3"], inputs["beta3"])


def _fallback_np(inputs):
    x = np.asarray(inputs["x"], np.float32)
    q = x @ inputs["Wq"] + inputs["bq"]
    k = x @ inputs["Wk"] + inputs["bk"]
    v = x @ inputs["Wv"] + inputs["bv"]

    def heads(t):
        return t.reshape(B, S, H, DK).transpose(0, 2, 1, 3)
    q, k, v = heads(q), heads(k), heads(v)
    sc = np.einsum('bhqd,bhkd->bhqk', q, k).astype(np.float32) / 8.0
    i = np.arange(S)[:, None]
    j = np.arange(S)[None, :]
    sc = np.where((j < i)[None, None], sc, -1e32)
    sc -= sc.max(-1, keepdims=True)
    e = np.exp(sc)
    att = e / e.sum(-1, keepdims=True)
    att = att * (i > 0)[None, None]
    ctx = np.einsum('bhqk,bhkd->bhqd', att, v)
    concat = ctx.transpose(0, 2, 1, 3).reshape(B, S, D)
    h1 = _ln_np(x + concat @ inputs["Wo"] + inputs["bo"],
                inputs["g1"], inputs["beta1"])
    f1 = np.maximum(h1 @ inputs["W1"] + inputs["b1"], 0.0)
    out = _ln_np(h1 + f1 @ inputs["W2"] + inputs["b2"],
                 inputs["g3"], inputs["beta3"])
    return out.astype(np.float32)


def _fast_path_ok(inputs):
    z = lambda n: not np.any(np.asarray(inputs[n]))
    o = lambda n: np.all(np.asarray(inputs[n]) == 1.0)
    return (z("bq") and z("bk") and z("bv") and z("bo") and z("b1")
            and z("b2") and z("beta1") and z("beta3") and o("g1") and o("g3"))


def _split8(w):
    """fp8 hi + scaled lo decomposition of a weight matrix."""
    w = np.asarray(w, np.float32)
    hi = w.astype(NP_F8)
    lo8 = (8.0 * (w - hi.astype(np.float32))).astype(NP_F8)
    return hi, lo8


def kernel(**inputs):
    x = np.asarray(inputs["x"], dtype=np.float32)
    assert x.shape == (B, S, D)
    if not _fast_path_ok(inputs):
        return _fallback_np(inputs)

    f8 = lambda a: np.ascontiguousarray(np.asarray(a)).astype(NP_F8)
    wvh, wvl = _split8(inputs["Wv"])
    woh, wol = _split8(inputs["Wo"])
    w1 = np.asarray(inputs["W1"], np.float32)
    w1t = np.ascontiguousarray(
        w1.reshape(DC, P, FC, P).transpose(2, 1, 0, 3))
    w1h = w1t.astype(NP_F8)
    w1l = (8.0 * (w1t - w1h.astype(np.float32))).astype(NP_F8)
    w2h, w2l = _split8(inputs["W2"])
    common = dict(
        wq8=f8(inputs["Wq"]), wk8=f8(inputs["Wk"]),
        wvh=wvh, wvl=wvl, woh=woh, wol=wol,
        w1h=w1h, w1l=w1l, w2h=w2h, w2l=w2l,
        mask8=np.triu(np.ones((P, P), np.float32), k=1).astype(NP_F8),
        ident=np.eye(P, dtype=np.float32),
        onesf=np.ones((P, 64), np.float32),
    )
    in_maps = []
    for i in range(B):
        xi = np.ascontiguousarray(x[i])
        xiT = np.ascontiguousarray(xi.T)
        xhi = xiT.astype(NP_F8)
        xlo = (xiT - xhi.astype(np.float32)).astype(NP_F8)
        xh8 = (xhi.astype(np.float32) / 8.0).astype(NP_F8)
        in_maps.append(dict(common, x32=xi, xT8=xhi, xTl8=xlo, xTh8=xh8))
    nc = _get_prog()
    res = run_bass_kernel_spmd(nc, in_maps, list(range(B)))
    out = np.stack([res.results[i]["out"] for i in range(B)], axis=0)
    out[:, 0, :] = _host_row0(inputs)
    return out


# revision 28
# speedup vs baseline: 1.1526x; 1.0193x over previous
"""Trainium2 Bass kernel for nn_BasicBlock (dense transformer block).

Data-parallel over batch: B=8 batch elements, one per NeuronCore, no
collectives. fp8e4m3-DoubleRow on every GEMM that tolerates it, with
scaled hi/lo error-correction splits where single fp8 would break the
2e-2 budget.

Numerics scheme (validated in numpy prototype, proto3.py):
  - A GEMM a@W in "sbs" (split-both-scaled) form runs three fp8-DR GEMMs
    accumulating into one PSUM: a_hi@W_hi + a_lo@W_hi + (a_hi/8)@(8*W_lo).
    The x8/ :8 scaling keeps the W-residual above e4m3's subnormal floor
    (W ~ 0.02 scale would otherwise flush to zero). Residual error ~0.1%.
  - FFN1, FFN2, V-proj: sbs (exact-ish). Out-proj: sws (W-corrected,
    concat single-quant). Scores Q/K bf16 storage (stage A), BMM2 in fp8
    DR with paired key-chunks (K=256/instruction).
  - Predicted rel err ~ 8e-3 vs the 2e-2 budget.

Per-core layout (S=1024, D=1024, H=16, d_k=64, d_ff=4096):
  - Q/K projections fp8 DoubleRow as before; scoresT bf16 per
    (head, key-chunk), causal columns only; exp on ScalarE writes fp8
    expT; strict lower-triangle mask via diagonal-strided DVE multiply;
    BMM2 runs DR over key-chunk PAIRS (the pre-zeroed odd-j gap regions
    make the paired reads causal-safe); ones-augmented V accumulates the
    softmax denominator; normalize fused into the fp8 quantize multiply.
  - Attention row 0 (zero-pad row) produces NaNs on device; host
    recomputes output row 0 exactly in numpy.
  - Fast path assumes the reference's structural constants (biases 0,
    gammas 1, betas 0); other inputs fall back to numpy.
"""

import numpy as np
import ml_dtypes
import concourse.bass as bass
import concourse.tile as tile
from concourse import bacc, mybir
from concourse.bass_utils import run_bass_kernel_spmd

F32 = mybir.dt.float32
F32R = mybir.dt.float32r
F8 = mybir.dt.float8e4
BF16 = mybir.dt.bfloat16
AF = mybir.ActivationFunctionType
OP = mybir.AluOpType
DR = mybir.MatmulPerfMode.DoubleRow

NP_F8 = ml_dtypes.float8_e4m3
NP_BF16 = ml_dtypes.bfloat16

B, S, D, H, DK, DFF = 8, 1024, 1024, 16, 64, 4096
P = 128
DC = D // P       # 8 chunks of d_model
DP = DC // 2      # 4 K-pairs of d_model (DR)
FC = DFF // P     # 32 chunks of d_ff
FP2 = FC // 2     # 16 K-pairs of d_ff (DR)
SC = S // P       # 8 chunks of sequence
EPS = 1e-5
SCALE = 0.125     # 1/sqrt(DK)


def _build():
    nc = bacc.Bacc("TRN2", target_bir_lowering=False, debug=False, num_devices=B)

    # per-batch inputs
    xT_d = nc.dram_tensor("xT8", [D, S], F8, kind="ExternalInput").ap()
    xTl_d = nc.dram_tensor("xTl8", [D, S], F8, kind="ExternalInput").ap()
    xTh8_d = nc.dram_tensor("xTh8", [D, S], F8, kind="ExternalInput").ap()
    x_d = nc.dram_tensor("x32", [S, D], F32, kind="ExternalInput").ap()
    # common weights
    wq_d = nc.dram_tensor("wq8", [D, D], F8, kind="ExternalInput").ap()
    wk_d = nc.dram_tensor("wk8", [D, D], F8, kind="ExternalInput").ap()
    wvh_d = nc.dram_tensor("wvh", [D, D], F8, kind="ExternalInput").ap()
    wvl_d = nc.dram_tensor("wvl", [D, D], F8, kind="ExternalInput").ap()
    woh_d = nc.dram_tensor("woh", [D, D], F8, kind="ExternalInput").ap()
    wol_d = nc.dram_tensor("wol", [D, D], F8, kind="ExternalInput").ap()
    # host pre-arranged: w1p[c, p, t, f] = W1[128t+p, 128c+f]
    w1h_d = nc.dram_tensor("w1h", [FC, P, DC, P], F8, kind="ExternalInput").ap()
    w1l_d = nc.dram_tensor("w1l", [FC, P, DC, P], F8, kind="ExternalInput").ap()
    w2h_d = nc.dram_tensor("w2h", [DFF, D], F8, kind="ExternalInput").ap()
    w2l_d = nc.dram_tensor("w2l", [DFF, D], F8, kind="ExternalInput").ap()
    mask_d = nc.dram_tensor("mask8", [P, P], F8, kind="ExternalInput").ap()
    id_d = nc.dram_tensor("ident", [P, P], F32, kind="ExternalInput").ap()
    ones_d = nc.dram_tensor("onesf", [P, 64], F32, kind="ExternalInput").ap()
    out_d = nc.dram_tensor("out", [S, D], F32, kind="ExternalOutput").ap()

    with tile.TileContext(nc) as tc:
      with tc.tile_pool(name="singles", bufs=1) as sg, \
           tc.tile_pool(name="sm", bufs=6) as sm:
        mask_rep = sg.tile([P, SC, P], F8)
        ident = sg.tile([P, P], F32)
        ones1r = sg.tile([P, 64], F32R)
        eps_t = sg.tile([P, 1], F32)
        h1 = sg.tile([P, SC, D], F32)
        h1T_hi = sg.tile([P, DC, S], F8)
        h1T_lo = sg.tile([P, DC, S], F8)
        h1T_h8 = sg.tile([P, DC, S], F8)

        nc.vector.memset(eps_t[:], EPS)
        nc.sync.dma_start(ident[:], id_d)
        nc.sync.dma_start(ones1r[:], ones_d.bitcast(F32R))
        # mask replicated 8x along a middle dim for the diagonal-AP multiply
        mask_bc = bass.AP(tensor=mask_d.tensor, offset=mask_d.offset,
                          ap=[[P, P], [0, SC], [1, P]])
        nc.sync.dma_start(mask_rep[:], mask_bc)

        # pool spanning V-proj .. out-proj/LN1
        midp_cm = tc.tile_pool(name="midp", bufs=1)
        mp_ = midp_cm.__enter__()
        concatT = mp_.tile([P, DC, S], F8)
        concat8 = mp_.tile([P, DC, S], F8)
        woh = mp_.tile([P, DC, D], F8)
        wol = mp_.tile([P, DC, D], F8)

        attp_cm = tc.tile_pool(name="attp", bufs=1)
        ap_ = attp_cm.__enter__()
        xT = ap_.tile([P, DC, S], F8)
        xTl = ap_.tile([P, DC, S], F8)
        xTh8 = ap_.tile([P, DC, S], F8)
        wq = ap_.tile([P, DC, D], F8)
        wk = ap_.tile([P, DC, D], F8)
        wvh = ap_.tile([P, DC, D], F8)
        wvl = ap_.tile([P, DC, D], F8)
        vaug = ap_.tile([P, SC, H, DK + 1], F8)

        # load order tuned for startup: x first, then the weights in the
        # order the first phases consume them (QK group0, V hi/lo/h8, ...)
        xT_r = xT_d.rearrange("(c p) s -> p c s", p=P)
        wq_r = wq_d.rearrange("(c p) d -> p c d", p=P)
        wk_r = wk_d.rearrange("(c p) d -> p c d", p=P)
        wvh_r = wvh_d.rearrange("(c p) d -> p c d", p=P)
        nc.sync.dma_start(xT[:, :, 0:256], xT_r[:, :, 0:256])
        nc.sync.dma_start(wq[:, :, 0:128], wq_r[:, :, 0:128])
        nc.sync.dma_start(wk[:, :, 0:128], wk_r[:, :, 0:128])
        nc.sync.dma_start(xT[:, :, 256:512], xT_r[:, :, 256:512])
        nc.sync.dma_start(xT[:, :, 512:768], xT_r[:, :, 512:768])
        nc.sync.dma_start(xT[:, :, 768:S], xT_r[:, :, 768:S])
        nc.sync.dma_start(wvh[:, 0:4, :], wvh_r[:, 0:4, :])
        nc.sync.dma_start(wvh[:, 4:8, :], wvh_r[:, 4:8, :])
        nc.sync.dma_start(xTl[:], xTl_d.rearrange("(c p) s -> p c s", p=P))
        nc.sync.dma_start(xTh8[:], xTh8_d.rearrange("(c p) s -> p c s", p=P))
        nc.sync.dma_start(wvl[:], wvl_d.rearrange("(c p) d -> p c d", p=P))
        nc.sync.dma_start(wq[:, :, 128:D], wq_r[:, :, 128:D])
        nc.sync.dma_start(wk[:, :, 128:D], wk_r[:, :, 128:D])
        nc.sync.dma_start(woh[:], woh_d.rearrange("(c p) d -> p c d", p=P))
        nc.sync.dma_start(wol[:], wol_d.rearrange("(c p) d -> p c d", p=P))
        nc.vector.memset(vaug[:, :, :, DK:DK + 1], 1.0)

        def ln_finish(stm, ap_full):
            # stm [P, 2, 6] bn_stats pairs; ap_full [P, 1024] in SBUF
            mv = sm.tile([P, 2], F32, tag="mv", name="mv")
            rs = sm.tile([P, 1], F32, tag="rs", name="rs")
            nb = sm.tile([P, 1], F32, tag="nb", name="nb")
            nc.vector.bn_aggr(mv[:], stm[:])
            nc.scalar.activation(rs[:], mv[:, 1:2], AF.Sqrt,
                                 bias=eps_t[:], scale=1.0)
            nc.vector.reciprocal(rs[:], rs[:])
            nc.vector.tensor_scalar(nb[:], mv[:, 0:1], rs[:], -1.0,
                                    op0=OP.mult, op1=OP.mult)
            nc.scalar.activation(ap_full, ap_full, AF.Identity,
                                 bias=nb[:], scale=rs[:])

        # ================= attention =================
        with tc.tile_pool(name="qkp", bufs=3) as qkp, \
             tc.tile_pool(name="expp", bufs=3) as expp, \
             tc.tile_pool(name="recp", bufs=1) as recp, \
             tc.tile_pool(name="psQK", bufs=2, space="PSUM") as psQK, \
             tc.tile_pool(name="psS", bufs=4, space="PSUM") as psS, \
             tc.tile_pool(name="psCD", bufs=1, space="PSUM") as psCD:

            # pre-zero the odd-j gap regions of the expT buffers:
            # (p, j=2jp+1, c in [256jp, 256jp+128)) is read by paired BMM2
            # but never written by exp.
            for i in range(3):
                e = expp.tile([P, SC, S], F8, tag="expT", name=f"expT{i}")
                base = e[:]
                gap = bass.AP(tensor=base.tensor, offset=base.offset + S,
                              ap=[base.ap[0], [2 * S + 256, 4], [1, P]])
                nc.vector.memset(gap, 0.0)

            pipe_qk = {}
            pipe_exp = {}
            pipe_cd = {}

            # Q/K projections: 2 heads packed per group g; q/k stored as
            # [64, 2, S] fp8 where partition 32*u2 + d32, plane i hold head
            # 2g+u2, d = d32 + 32*i (u2 head-in-group).  Wq/Wk are
            # host-permuted so a group's 64 output columns per d-half are
            # contiguous.  This layout lets the d_k=64-contraction score
            # matmuls run fp8 DoubleRow as [32, 2, *] APs (AP partition base
            # limited to 0/32/64, so at most 2 heads share a tile).
            def emit_qk_group(g, qk, quarter):
                if qk == 0 and quarter == 0:
                    qt = qkp.tile([64, 2, S], F8, tag="qT", name=f"qT{g}")
                    kt = qkp.tile([64, 2, S], F8, tag="kT", name=f"kT{g}")
                    pipe_qk[g] = (qt, kt)
                qt, kt = pipe_qk[g]
                dst = qt if qk == 0 else kt
                w = wq if qk == 0 else wk
                pq = psQK.tile([64, 2, 256], F32, tag="pq", name="pq")
                scol = 256 * quarter
                for i in range(2):
                    wcol = 128 * g + 64 * i
                    for t in range(4):
                        nc.tensor.matmul(
                            pq[:, i, :],
                            w[:, 2 * t:2 * t + 2, wcol:wcol + 64],
                            xT[:, 2 * t:2 * t + 2, scol:scol + 256],
                            start=(t == 0), stop=(t == 3),
                            perf_mode=DR, skip_group_check=True)
                dcols = dst[:, :, scol:scol + 256]
                nc.vector.tensor_copy(dcols, pq[:])

            def emit_scores_js(h, js):
                if js[0] == 0:
                    pipe_exp[h] = expp.tile([P, SC, S], F8, tag="expT",
                                            name=f"expT_h{h}")
                qt0, kt0 = pipe_qk[h // 2]
                u = 32 * (h % 2)
                qt = qt0[u:u + 32, :, :]
                expT = pipe_exp[h]
                for j in js:
                    lo = P * j
                    kslice = kt0[u:u + 32, :, lo:lo + P]
                    if lo < 512:
                        ps = psS.tile([P, 512], F32, tag="sc", name="sc")
                        nc.tensor.matmul(ps[:, lo:512], kslice,
                                         qt[:, :, lo:512],
                                         start=True, stop=True,
                                         perf_mode=DR, skip_group_check=True)
                        nc.scalar.activation(expT[:, j, lo:512], ps[:, lo:512],
                                             AF.Exp, bias=0.0, scale=SCALE)
                        ps2 = psS.tile([P, 512], F32, tag="sc", name="sc2")
                        nc.tensor.matmul(ps2[:], kslice, qt[:, :, 512:S],
                                         start=True, stop=True,
                                         perf_mode=DR, skip_group_check=True)
                        nc.scalar.activation(expT[:, j, 512:S], ps2[:],
                                             AF.Exp, bias=0.0, scale=SCALE)
                    else:
                        ps = psS.tile([P, 512], F32, tag="sc", name="sc")
                        nc.tensor.matmul(ps[:, lo - 512:512], kslice,
                                         qt[:, :, lo:S],
                                         start=True, stop=True,
                                         perf_mode=DR, skip_group_check=True)
                        nc.scalar.activation(expT[:, j, lo:S],
                                             ps[:, lo - 512:512],
                                             AF.Exp, bias=0.0, scale=SCALE)
                if js[-1] == SC - 1:
                    # strict-upper mask on the 8 diagonal blocks, one Pool op
                    if h % 2 == 1:
                        del pipe_qk[h // 2]
                    base = expT[:]
                    diag = bass.AP(tensor=base.tensor, offset=base.offset,
                                   ap=[base.ap[0], [S + P, SC], [1, P]])
                    mb = mask_rep[:]
                    mask3 = bass.AP(tensor=mb.tensor, offset=mb.offset,
                                    ap=[mb.ap[0], [P, SC], [1, P]])
                    nc.gpsimd.tensor_mul(diag, diag, mask3)

            # BMM2: fp8 DoubleRow over key-chunk pairs (K=256/instruction)
            def emit_bmm2_mm(h):
                expT = pipe_exp.pop(h)
                cds = []
                for n in range(2):
                    cd = psCD.tile([P, 512], F32, tag=f"cd{n}", name=f"cd{n}")
                    cds.append(cd)
                    # causal pairs for this column half
                    if n == 0:
                        pairs = [(0, 0), (2, 256)]
                    else:
                        pairs = [(0, 512), (2, 512), (4, 512), (6, 768)]
                    for pi, (j, c0) in enumerate(pairs):
                        nc.tensor.matmul(
                            cd[0:DK + 1, c0 - 512 * n:512],
                            vaug[:, j:j + 2, h, :],
                            expT[:, j:j + 2, c0:512 * (n + 1)],
                            start=(pi == 0), stop=(pi == len(pairs) - 1),
                            perf_mode=DR, skip_group_check=True)
                    rt = recp.tile([P, 512], F32R, tag="rec", name="rec")
                    nc.vector.tensor_copy(rt[DK:DK + 1, :], cd[DK:DK + 1, :])
                    cds.append(rt)
                pipe_cd[h] = cds

            def emit_bmm2_norm(h):
                cd0, rt0, cd1, rt1 = pipe_cd.pop(h)
                ch, off = h // 2, 64 * (h % 2)
                tmp = None
                if off:
                    tmp = recp.tile([64, S], F8, tag="tmp", name="tmp")
                for n, (cd, rt) in enumerate(((cd0, rt0), (cd1, rt1))):
                    # broadcast den across 64 partitions (PE), reciprocal to
                    # SBUF, normalize (single-PSUM-input multiply) with fp8
                    # quantize fused in.
                    pb = psS.tile([P, 512], F32, tag="sc", name="bc")
                    nc.tensor.matmul(pb[0:64, 0:512],
                                     ones1r[DK:DK + 1, :],
                                     rt[DK:DK + 1, :],
                                     start=True, stop=True,
                                     skip_group_check=True)
                    rb = recp.tile([64, 512], F32, tag="rb", name="rb")
                    nc.vector.reciprocal(rb[:], pb[0:64, 0:512])
                    dst = (concatT[0:64, ch, 512 * n:512 * (n + 1)]
                           if off == 0 else tmp[:, 512 * n:512 * (n + 1)])
                    nc.vector.tensor_mul(dst, cd[0:DK, :], rb[:])
                if off:
                    nc.sync.dma_start(concatT[64:P, ch, :], tmp[:])
                    # concat/8 for the Wo-residual correction GEMM (sws)
                    nc.gpsimd.tensor_scalar_mul(concat8[:, ch, :],
                                                concatT[:, ch, :], 0.125)

            for part in range(8):
                emit_qk_group(0, part // 4, part % 4)

            # ============ phase V: V projection (sbs, 3 DR GEMMs) ===========
            # interleaved with the first two heads' score matmuls so the
            # Scalar engine starts exp'ing while the PE runs V.
            def emit_v(m, nf):
                ps = psS.tile([P, 512], F32, tag="sc", name="v")[:, :]
                ncol = slice(512 * nf, 512 * (nf + 1))
                mcol = slice(P * m, P * (m + 1))
                for t in range(DP):
                    tp = slice(2 * t, 2 * t + 2)
                    nc.tensor.matmul(
                        ps, xT[:, tp, mcol], wvh[:, tp, ncol],
                        start=(t == 0), stop=False,
                        perf_mode=DR, skip_group_check=True)
                for t in range(DP):
                    tp = slice(2 * t, 2 * t + 2)
                    nc.tensor.matmul(
                        ps, xTl[:, tp, mcol], wvh[:, tp, ncol],
                        start=False, stop=False,
                        perf_mode=DR, skip_group_check=True)
                for t in range(DP):
                    tp = slice(2 * t, 2 * t + 2)
                    nc.tensor.matmul(
                        ps, xTh8[:, tp, mcol], wvl[:, tp, ncol],
                        start=False, stop=(t == DP - 1),
                        perf_mode=DR, skip_group_check=True)
                nc.scalar.activation(
                    vaug[:, m, 8 * nf:8 * (nf + 1), 0:DK],
                    ps.rearrange("p (h d) -> p h d", d=DK),
                    AF.Identity)

            emit_scores_js(0, [0, 1])
            for m in range(SC // 2):
                emit_v(m, 0)
                emit_v(m, 1)
            emit_scores_js(0, [2, 3])
            emit_qk_group(1, 0, 0)
            emit_scores_js(0, [4, 5, 6, 7])
            for m in range(SC // 2, SC):
                if m < SC - 1:
                    emit_qk_group(1, 0, m - 3)
                emit_v(m, 0)
                emit_v(m, 1)

            for h in range(2, H + 2):
                hh = h - 1
                g, u2 = hh // 2, hh % 2
                nxt = g + 1 < H // 2 and h <= H

                def qkc(ci):
                    c = 4 * u2 + ci
                    emit_qk_group(g + 1, c // 4, c % 4)

                if h <= H:
                    emit_scores_js(hh, [0, 1])
                if nxt:
                    qkc(0)
                if h >= 2:
                    emit_bmm2_mm(h - 2)
                if nxt:
                    qkc(1)
                if h <= H:
                    emit_scores_js(hh, [2, 3])
                if nxt:
                    qkc(2)
                if h >= 2:
                    emit_bmm2_norm(h - 2)
                if h <= H:
                    emit_scores_js(hh, [4, 5, 6, 7])
                if nxt:
                    qkc(3)

        attp_cm.__exit__(None, None, None)

        # ===== phase C/D: out-proj (sws, 2 DR GEMMs) + residual + LN1 + T ====
        with tc.tile_pool(name="xs", bufs=4) as xs, \
             tc.tile_pool(name="psC", bufs=4, space="PSUM") as psC, \
             tc.tile_pool(name="psT", bufs=3, space="PSUM") as psT:
            def emit_transpose(m):
                for cq in range(2):
                    pt = psT.tile([P, 4, P], F32, tag="t", name="t")
                    for ci in range(4):
                        c = 4 * cq + ci
                        nc.tensor.matmul(
                            pt[:, ci, :],
                            h1[:, m, P * c:P * (c + 1)],
                            ident[:], is_transpose=True,
                            start=True, stop=True, skip_group_check=True)
                    cols = slice(P * m, P * (m + 1))
                    cq4 = slice(4 * cq, 4 * (cq + 1))
                    hi = h1T_hi[:, cq4, cols]
                    nc.scalar.activation(hi, pt[:], AF.Identity)
                    nc.vector.scalar_tensor_tensor(
                        h1T_lo[:, cq4, cols], pt[:], 1.0, hi,
                        op0=OP.mult, op1=OP.subtract)
                    nc.gpsimd.tensor_scalar_mul(
                        h1T_h8[:, cq4, cols], hi, 0.125)

            for m in range(SC + 2):
                if m < SC:
                    xm = xs.tile([P, D], F32, tag="x", name=f"x{m}")
                    nc.sync.dma_start(xm[:], x_d[P * m:P * (m + 1), :])
                    stm = sm.tile([P, 2, 6], F32, tag="st", name="st")
                    mcol = slice(P * m, P * (m + 1))
                    for n in range(2):
                        pc = psC.tile([P, 512], F32, tag="c", name="c")
                        ncol = slice(512 * n, 512 * (n + 1))
                        for c in range(DP):
                            cp = slice(2 * c, 2 * c + 2)
                            nc.tensor.matmul(
                                pc[:], concatT[:, cp, mcol], woh[:, cp, ncol],
                                start=(c == 0), stop=False,
                                perf_mode=DR, skip_group_check=True)
                        for c in range(DP):
                            cp = slice(2 * c, 2 * c + 2)
                            nc.tensor.matmul(
                                pc[:], concat8[:, cp, mcol], wol[:, cp, ncol],
                                start=False, stop=(c == DP - 1),
                                perf_mode=DR, skip_group_check=True)
                        nc.vector.tensor_add(h1[:, m, ncol], pc[:], xm[:, ncol])
                        nc.vector.bn_stats(stm[:, n, :], h1[:, m, ncol])
                    ln_finish(stm, h1[:, m, :])
                if m >= 2:
                    emit_transpose(m - 2)

        midp_cm.__exit__(None, None, None)

        # ================= phase E: FFN1 (sbs, 3 DR GEMMs) =================
        ftp_cm = tc.tile_pool(name="ftp", bufs=1)
        ftp = ftp_cm.__enter__()
        fT_hi = ftp.tile([P, FC, S], F8)
        fT_lo = ftp.tile([P, FC, S], F8)
        fT_h8 = ftp.tile([P, FC, S], F8)
        with tc.tile_pool(name="w1s", bufs=6) as w1s, \
             tc.tile_pool(name="psF", bufs=2, space="PSUM") as psF:
            for c in range(FC):
                wth = w1s.tile([P, DC, P], F8, tag="w1h", name=f"w1h_{c}")
                wtl = w1s.tile([P, DC, P], F8, tag="w1l", name=f"w1l_{c}")
                nc.sync.dma_start(wth[:], w1h_d[c])
                nc.sync.dma_start(wtl[:], w1l_d[c])
                pf = psF.tile([P, 2, 512], F32, tag="f", name="f")
                for sh in range(2):
                    cols = slice(512 * sh, 512 * (sh + 1))
                    for t in range(DP):
                        tp = slice(2 * t, 2 * t + 2)
                        nc.tensor.matmul(
                            pf[:, sh, :], wth[:, tp, :], h1T_hi[:, tp, cols],
                            start=(t == 0), stop=False,
                            perf_mode=DR, skip_group_check=True)
                    for t in range(DP):
                        tp = slice(2 * t, 2 * t + 2)
                        nc.tensor.matmul(
                            pf[:, sh, :], wth[:, tp, :], h1T_lo[:, tp, cols],
                            start=False, stop=False,
                            perf_mode=DR, skip_group_check=True)
                    for t in range(DP):
                        tp = slice(2 * t, 2 * t + 2)
                        nc.tensor.matmul(
                            pf[:, sh, :], wtl[:, tp, :], h1T_h8[:, tp, cols],
                            start=False, stop=(t == DP - 1),
                            perf_mode=DR, skip_group_check=True)
                pff = pf[:].rearrange("p a b -> p (a b)")
                nc.scalar.activation(fT_hi[:, c, :], pff, AF.Relu)
                nc.vector.scalar_tensor_tensor(
                    fT_lo[:, c, :], pff, 0.0, fT_hi[:, c, :],
                    op0=OP.max, op1=OP.subtract)
                nc.gpsimd.tensor_scalar_mul(fT_h8[:, c, :], fT_hi[:, c, :],
                                            0.125)

        # ============ phase F: FFN2 (sbs, 3 DR GEMMs) + LN2 + out ===========
        w2h_r = w2h_d.rearrange("(c p) d -> p c d", p=P)
        w2l_r = w2l_d.rearrange("(c p) d -> p c d", p=P)
        with tc.tile_pool(name="w2s", bufs=4) as w2s, \
             tc.tile_pool(name="psG", bufs=1, space="PSUM") as psG:
            wts = {}

            def w2dma(i):
                half, c2 = divmod(i, FP2)
                wh = w2s.tile([P, 2, D], F8, tag="w2h", name=f"w2h_{half}_{c2}")
                wl = w2s.tile([P, 2, D], F8, tag="w2l", name=f"w2l_{half}_{c2}")
                cp = slice(2 * c2, 2 * c2 + 2)
                nc.sync.dma_start(wh[:], w2h_r[:, cp, :])
                nc.sync.dma_start(wl[:], w2l_r[:, cp, :])
                wts[i] = (wh, wl)

            w2dma(0)
            w2dma(1)
            for half in range(2):
                gt = [[psG.tile([P, 512], F32, tag=f"g{mi}{n}",
                                name=f"g{half}{mi}{n}")
                       for n in range(2)] for mi in range(4)]
                for c2 in range(FP2):
                    i = half * FP2 + c2
                    if i + 2 < 2 * FP2:
                        w2dma(i + 2)
                    wh, wl = wts.pop(i)
                    cp = slice(2 * c2, 2 * c2 + 2)
                    for mi in range(4):
                        m = 4 * half + mi
                        mcol = slice(P * m, P * (m + 1))
                        for n in range(2):
                            ncol = slice(512 * n, 512 * (n + 1))
                            nc.tensor.matmul(
                                gt[mi][n][:], fT_hi[:, cp, mcol],
                                wh[:, :, ncol],
                                start=(c2 == 0), stop=False,
                                perf_mode=DR, skip_group_check=True)
                            nc.tensor.matmul(
                                gt[mi][n][:], fT_lo[:, cp, mcol],
                                wh[:, :, ncol],
                                start=False, stop=False,
                                perf_mode=DR, skip_group_check=True)
                            nc.tensor.matmul(
                                gt[mi][n][:], fT_h8[:, cp, mcol],
                                wl[:, :, ncol],
                                start=False, stop=(c2 == FP2 - 1),
                                perf_mode=DR, skip_group_check=True)
                for mi in range(4):
                    m = 4 * half + mi
                    stm = sm.tile([P, 2, 6], F32, tag="st", name="st2")
                    for n in range(2):
                        cols = slice(512 * n, 512 * (n + 1))
                        nc.vector.tensor_add(h1[:, m, cols], gt[mi][n][:],
                                             h1[:, m, cols])
                        nc.vector.bn_stats(stm[:, n, :], h1[:, m, cols])
                    ln_finish(stm, h1[:, m, :])
                    nc.sync.dma_start(out_d[P * m:P * (m + 1), :], h1[:, m, :])
        ftp_cm.__exit__(None, None, None)

    nc.compile()
    return nc


_cached = None


def _get_prog():
    global _cached
    if _cached is None:
        _cached = _build()
    return _cached


def _ln_np(v, g, b):
    mu = v.mean(-1, keepdims=True)
    var = v.var(-1, keepdims=True)
    return (v - mu) / np.sqrt(var + EPS) * g + b


def _host_row0(inputs):
    """Exact f32 recompute of output row 0 (attention row 0 is zero-padded)."""
    x0 = np.asarray(inputs["x"], np.float32)[:, 0, :]          # [B, D]
    h1 = _ln_np(x0 + inputs["bo"][None, :], inputs["g1"], inputs["beta1"])
    f1 = np.maximum(h1 @ inputs["W1"] + inputs["b1"], 0.0)
    ffn = f1 @ inputs["W2"] + inputs["b2"]
    return _ln_np(h1 + ffn, inputs["g3"], inputs["beta3"])


def _fallback_np(inputs):
    x = np.asarray(inputs["x"], np.float32)
    q = x @ inputs["Wq"] + inputs["bq"]
    k = x @ inputs["Wk"] + inputs["bk"]
    v = x @ inputs["Wv"] + inputs["bv"]

    def heads(t):
        return t.reshape(B, S, H, DK).transpose(0, 2, 1, 3)
    q, k, v = heads(q), heads(k), heads(v)
    sc = np.einsum('bhqd,bhkd->bhqk', q, k).astype(np.float32) / 8.0
    i = np.arange(S)[:, None]
    j = np.arange(S)[None, :]
    sc = np.where((j < i)[None, None], sc, -1e32)
    sc -= sc.max(-1, keepdims=True)
    e = np.exp(sc)
    att = e / e.sum(-1, keepdims=True)
    att = att * (i > 0)[None, None]
    ctx = np.einsum('bhqk,bhkd->bhqd', att, v)
    concat = ctx.transpose(0, 2, 1, 3).reshape(B, S, D)
    h1 = _ln_np(x + concat @ inputs["Wo"] + inputs["bo"],
                inputs["g1"], inputs["beta1"])
    f1 = np.maximum(h1 @ inputs["W1"] + inputs["b1"], 0.0)
    out = _ln_np(h1 + f1 @ inputs["W2"] + inputs["b2"],
                 inputs["g3"], inputs["beta3"])
    return out.astype(np.float32)


def _fast_path_ok(inputs):
    z = lambda n: not np.any(np.asarray(inputs[n]))
    o = lambda n: np.all(np.asarray(inputs[n]) == 1.0)
    return (z("bq") and z("bk") and z("bv") and z("bo") and z("b1")
            and z("b2") and z("beta1") and z("beta3") and o("g1") and o("g3"))


def _split8(w):
    """fp8 hi + scaled lo decomposition of a weight matrix."""
    w = np.asarray(w, np.float32)
    hi = w.astype(NP_F8)
    lo8 = (8.0 * (w - hi.astype(np.float32))).astype(NP_F8)
    return hi, lo8


def kernel(**inputs):
    x = np.asarray(inputs["x"], dtype=np.float32)
    assert x.shape == (B, S, D)
    if not _fast_path_ok(inputs):
        return _fallback_np(inputs)

    f8 = lambda a: np.ascontiguousarray(np.asarray(a)).astype(NP_F8)
    # permute Wq/Wk columns so each 2-head group's 64 projection outputs
    # per d-half (heads 2g+u2, d-half i) are contiguous: col
    # 128g+64i+32u2+d32 maps to original head 2g+u2, d = 32i + d32.
    col = np.arange(D)
    g_, r_ = np.divmod(col, 128)
    i_, r2_ = np.divmod(r_, 64)
    u_, d_ = np.divmod(r2_, 32)
    perm = 64 * (2 * g_ + u_) + 32 * i_ + d_
    wvh, wvl = _split8(inputs["Wv"])
    woh, wol = _split8(inputs["Wo"])
    w1 = np.asarray(inputs["W1"], np.float32)
    w1t = np.ascontiguousarray(
        w1.reshape(DC, P, FC, P).transpose(2, 1, 0, 3))
    w1h = w1t.astype(NP_F8)
    w1l = (8.0 * (w1t - w1h.astype(np.float32))).astype(NP_F8)
    w2h, w2l = _split8(inputs["W2"])
    common = dict(
        wq8=f8(np.asarray(inputs["Wq"])[:, perm]),
        wk8=f8(np.asarray(inputs["Wk"])[:, perm]),
        wvh=wvh, wvl=wvl, woh=woh, wol=wol,
        w1h=w1h, w1l=w1l, w2h=w2h, w2l=w2l,
        mask8=np.triu(np.ones((P, P), np.float32), k=1).astype(NP_F8),
        ident=np.eye(P, dtype=np.float32),
        onesf=np.ones((P, 64), np.float32),
    )
    in_maps = []
    for i in range(B):
        xi = np.ascontiguousarray(x[i])
        xiT = np.ascontiguousarray(xi.T)
        xhi = xiT.astype(NP_F8)
        xlo = (xiT - xhi.astype(np.float32)).astype(NP_F8)
        xh8 = (xhi.astype(np.float32) / 8.0).astype(NP_F8)
        in_maps.append(dict(common, x32=xi, xT8=xhi, xTl8=xlo, xTh8=xh8))
    nc = _get_prog()
    res = run_bass_kernel_spmd(nc, in_maps, list(range(B)))
    out = np.stack([res.results[i]["out"] for i in range(B)], axis=0)
    out[:, 0, :] = _host_row0(inputs)
    return out


# revision 34
# speedup vs baseline: 1.1709x; 1.0158x over previous
"""Trainium2 Bass kernel for nn_BasicBlock (dense transformer block).

Data-parallel over batch: B=8 batch elements, one per NeuronCore, no
collectives. fp8e4m3-DoubleRow on every GEMM that tolerates it, with
scaled hi/lo error-correction splits where single fp8 would break the
2e-2 budget.

Numerics scheme (validated in numpy prototype, proto3.py):
  - A GEMM a@W in "sbs" (split-both-scaled) form runs three fp8-DR GEMMs
    accumulating into one PSUM: a_hi@W_hi + a_lo@W_hi + (a_hi/8)@(8*W_lo).
    The x8/ :8 scaling keeps the W-residual above e4m3's subnormal floor
    (W ~ 0.02 scale would otherwise flush to zero). Residual error ~0.1%.
  - FFN1, FFN2, V-proj: sbs (exact-ish). Out-proj: sws (W-corrected,
    concat single-quant). Scores Q/K bf16 storage (stage A), BMM2 in fp8
    DR with paired key-chunks (K=256/instruction).
  - Predicted rel err ~ 8e-3 vs the 2e-2 budget.

Per-core layout (S=1024, D=1024, H=16, d_k=64, d_ff=4096):
  - Q/K projections fp8 DoubleRow as before; scoresT bf16 per
    (head, key-chunk), causal columns only; exp on ScalarE writes fp8
    expT; strict lower-triangle mask via diagonal-strided DVE multiply;
    BMM2 runs DR over key-chunk PAIRS (the pre-zeroed odd-j gap regions
    make the paired reads causal-safe); ones-augmented V accumulates the
    softmax denominator; normalize fused into the fp8 quantize multiply.
  - Attention row 0 (zero-pad row) produces NaNs on device; host
    recomputes output row 0 exactly in numpy.
  - Fast path assumes the reference's structural constants (biases 0,
    gammas 1, betas 0); other inputs fall back to numpy.
"""

import numpy as np
import ml_dtypes
import concourse.bass as bass
import concourse.tile as tile
from concourse import bacc, mybir
from concourse.bass_utils import run_bass_kernel_spmd

F32 = mybir.dt.float32
F32R = mybir.dt.float32r
F8 = mybir.dt.float8e4
BF16 = mybir.dt.bfloat16
AF = mybir.ActivationFunctionType
OP = mybir.AluOpType
DR = mybir.MatmulPerfMode.DoubleRow

NP_F8 = ml_dtypes.float8_e4m3
NP_BF16 = ml_dtypes.bfloat16

B, S, D, H, DK, DFF = 8, 1024, 1024, 16, 64, 4096
P = 128
DC = D // P       # 8 chunks of d_model
DP = DC // 2      # 4 K-pairs of d_model (DR)
FC = DFF // P     # 32 chunks of d_ff
FP2 = FC // 2     # 16 K-pairs of d_ff (DR)
SC = S // P       # 8 chunks of sequence
EPS = 1e-5
SCALE = 0.125     # 1/sqrt(DK)


def _build():
    nc = bacc.Bacc("TRN2", target_bir_lowering=False, debug=False, num_devices=B)

    # per-batch inputs
    xT_d = nc.dram_tensor("xT8", [D, S], F8, kind="ExternalInput").ap()
    xTl_d = nc.dram_tensor("xTl8", [D, S], F8, kind="ExternalInput").ap()
    xTh8_d = nc.dram_tensor("xTh8", [D, S], F8, kind="ExternalInput").ap()
    x_d = nc.dram_tensor("x32", [S, D], F32, kind="ExternalInput").ap()
    # common weights
    wq_d = nc.dram_tensor("wq8", [D, D], F8, kind="ExternalInput").ap()
    wk_d = nc.dram_tensor("wk8", [D, D], F8, kind="ExternalInput").ap()
    wvh_d = nc.dram_tensor("wvh", [D, D], F8, kind="ExternalInput").ap()
    wvl_d = nc.dram_tensor("wvl", [D, D], F8, kind="ExternalInput").ap()
    woh_d = nc.dram_tensor("woh", [D, D], F8, kind="ExternalInput").ap()
    wol_d = nc.dram_tensor("wol", [D, D], F8, kind="ExternalInput").ap()
    # host pre-arranged: w1p[c, p, t, f] = W1[128t+p, 128c+f]
    w1h_d = nc.dram_tensor("w1h", [FC, P, DC, P], F8, kind="ExternalInput").ap()
    w1l_d = nc.dram_tensor("w1l", [FC, P, DC, P], F8, kind="ExternalInput").ap()
    w2h_d = nc.dram_tensor("w2h", [DFF, D], F8, kind="ExternalInput").ap()
    w2l_d = nc.dram_tensor("w2l", [DFF, D], F8, kind="ExternalInput").ap()
    mask_d = nc.dram_tensor("mask8", [P, P], F8, kind="ExternalInput").ap()
    id_d = nc.dram_tensor("ident", [P, P], F32, kind="ExternalInput").ap()
    ones_d = nc.dram_tensor("onesf", [P, 64], F32, kind="ExternalInput").ap()
    out_d = nc.dram_tensor("out", [S, D], F32, kind="ExternalOutput").ap()

    with tile.TileContext(nc) as tc:
      with tc.tile_pool(name="singles", bufs=1) as sg, \
           tc.tile_pool(name="sm", bufs=6) as sm:
        mask_rep = sg.tile([P, SC, P], F8)
        ident = sg.tile([P, P], F32)
        ones1r = sg.tile([P, 64], F32R)
        eps_t = sg.tile([P, 1], F32)
        h1 = sg.tile([P, SC, D], F32)
        h1T_hi = sg.tile([P, DC, S], F8)
        h1T_lo = sg.tile([P, DC, S], F8)
        h1T_h8 = sg.tile([P, DC, S], F8)

        nc.vector.memset(eps_t[:], EPS)
        nc.sync.dma_start(ident[:], id_d)
        nc.sync.dma_start(ones1r[:], ones_d.bitcast(F32R))
        # mask replicated 8x along a middle dim for the diagonal-AP multiply
        mask_bc = bass.AP(tensor=mask_d.tensor, offset=mask_d.offset,
                          ap=[[P, P], [0, SC], [1, P]])
        nc.sync.dma_start(mask_rep[:], mask_bc)

        # pool spanning V-proj .. out-proj/LN1
        midp_cm = tc.tile_pool(name="midp", bufs=1)
        mp_ = midp_cm.__enter__()
        concatT = mp_.tile([P, DC, S], F8)
        concat8 = mp_.tile([P, DC, S], F8)
        woh = mp_.tile([P, DC, D], F8)
        wol = mp_.tile([P, DC, D], F8)

        attp_cm = tc.tile_pool(name="attp", bufs=1)
        ap_ = attp_cm.__enter__()
        xT = ap_.tile([P, DC, S], F8)
        xTl = ap_.tile([P, DC, S], F8)
        xTh8 = ap_.tile([P, DC, S], F8)
        wq = ap_.tile([P, DC, D], F8)
        wk = ap_.tile([P, DC, D], F8)
        wvh = ap_.tile([P, DC, D], F8)
        wvl = ap_.tile([P, DC, D], F8)
        vaug = ap_.tile([P, SC, H, DK + 1], F8)

        # load order tuned for startup: x first, then the weights in the
        # order the first phases consume them (QK group0, V hi/lo/h8, ...)
        xT_r = xT_d.rearrange("(c p) s -> p c s", p=P)
        wq_r = wq_d.rearrange("(c p) d -> p c d", p=P)
        wk_r = wk_d.rearrange("(c p) d -> p c d", p=P)
        wvh_r = wvh_d.rearrange("(c p) d -> p c d", p=P)
        nc.sync.dma_start(xT[:, :, 0:256], xT_r[:, :, 0:256])
        nc.sync.dma_start(wq[:, :, 0:128], wq_r[:, :, 0:128])
        nc.sync.dma_start(wk[:, :, 0:128], wk_r[:, :, 0:128])
        nc.sync.dma_start(xT[:, :, 256:512], xT_r[:, :, 256:512])
        nc.sync.dma_start(xT[:, :, 512:768], xT_r[:, :, 512:768])
        nc.sync.dma_start(xT[:, :, 768:S], xT_r[:, :, 768:S])
        nc.sync.dma_start(wvh[:, 0:4, :], wvh_r[:, 0:4, :])
        nc.sync.dma_start(wvh[:, 4:8, :], wvh_r[:, 4:8, :])
        nc.sync.dma_start(xTl[:], xTl_d.rearrange("(c p) s -> p c s", p=P))
        nc.sync.dma_start(xTh8[:], xTh8_d.rearrange("(c p) s -> p c s", p=P))
        nc.sync.dma_start(wvl[:], wvl_d.rearrange("(c p) d -> p c d", p=P))
        nc.sync.dma_start(wq[:, :, 128:D], wq_r[:, :, 128:D])
        nc.sync.dma_start(wk[:, :, 128:D], wk_r[:, :, 128:D])
        nc.sync.dma_start(woh[:], woh_d.rearrange("(c p) d -> p c d", p=P))
        nc.sync.dma_start(wol[:], wol_d.rearrange("(c p) d -> p c d", p=P))
        nc.vector.memset(vaug[:, :, :, DK:DK + 1], 1.0)

        def ln_finish(stm, ap_full):
            # stm [P, 2, 6] bn_stats pairs; ap_full [P, 1024] in SBUF
            mv = sm.tile([P, 2], F32, tag="mv", name="mv")
            rs = sm.tile([P, 1], F32, tag="rs", name="rs")
            nb = sm.tile([P, 1], F32, tag="nb", name="nb")
            nc.vector.bn_aggr(mv[:], stm[:])
            nc.scalar.activation(rs[:], mv[:, 1:2], AF.Sqrt,
                                 bias=eps_t[:], scale=1.0)
            nc.vector.reciprocal(rs[:], rs[:])
            nc.vector.tensor_scalar(nb[:], mv[:, 0:1], rs[:], -1.0,
                                    op0=OP.mult, op1=OP.mult)
            nc.scalar.activation(ap_full, ap_full, AF.Identity,
                                 bias=nb[:], scale=rs[:])

        # ================= attention =================
        with tc.tile_pool(name="qkp", bufs=3) as qkp, \
             tc.tile_pool(name="expp", bufs=3) as expp, \
             tc.tile_pool(name="recp", bufs=1) as recp, \
             tc.tile_pool(name="psQK", bufs=2, space="PSUM") as psQK, \
             tc.tile_pool(name="psS", bufs=4, space="PSUM") as psS, \
             tc.tile_pool(name="psCD", bufs=1, space="PSUM") as psCD:

            # pre-zero the odd-j gap regions of the expT buffers:
            # (p, j=2jp+1, c in [256jp, 256jp+128)) is read by paired BMM2
            # but never written by exp.
            for i in range(3):
                e = expp.tile([P, SC, S], F8, tag="expT", name=f"expT{i}")
                base = e[:]
                gap = bass.AP(tensor=base.tensor, offset=base.offset + S,
                              ap=[base.ap[0], [2 * S + 256, 4], [1, P]])
                nc.vector.memset(gap, 0.0)

            pipe_qk = {}
            pipe_exp = {}
            pipe_cd = {}

            # Q/K projections: 2 heads packed per group g; q/k stored as
            # [64, 2, S] fp8 where partition 32*u2 + d32, plane i hold head
            # 2g+u2, d = d32 + 32*i (u2 head-in-group).  Wq/Wk are
            # host-permuted so a group's 64 output columns per d-half are
            # contiguous.  This layout lets the d_k=64-contraction score
            # matmuls run fp8 DoubleRow as [32, 2, *] APs (AP partition base
            # limited to 0/32/64, so at most 2 heads share a tile).
            def emit_qk_group(g, qk, quarter):
                if qk == 0 and quarter == 0:
                    qt = qkp.tile([64, 2, S], F8, tag="qT", name=f"qT{g}")
                    kt = qkp.tile([64, 2, S], F8, tag="kT", name=f"kT{g}")
                    pipe_qk[g] = (qt, kt)
                qt, kt = pipe_qk[g]
                dst = qt if qk == 0 else kt
                w = wq if qk == 0 else wk
                pq = psQK.tile([64, 2, 256], F32, tag="pq", name="pq")
                scol = 256 * quarter
                for i in range(2):
                    wcol = 128 * g + 64 * i
                    for t in range(4):
                        nc.tensor.matmul(
                            pq[:, i, :],
                            w[:, 2 * t:2 * t + 2, wcol:wcol + 64],
                            xT[:, 2 * t:2 * t + 2, scol:scol + 256],
                            start=(t == 0), stop=(t == 3),
                            perf_mode=DR, skip_group_check=True)
                dcols = dst[:, :, scol:scol + 256]
                nc.vector.tensor_copy(dcols, pq[:])

            def emit_scores_js(h, js):
                if js[0] == 0:
                    pipe_exp[h] = expp.tile([P, SC, S], F8, tag="expT",
                                            name=f"expT_h{h}")
                qt0, kt0 = pipe_qk[h // 2]
                u = 32 * (h % 2)
                qt = qt0[u:u + 32, :, :]
                expT = pipe_exp[h]
                for j in js:
                    lo = P * j
                    kslice = kt0[u:u + 32, :, lo:lo + P]
                    if lo < 512:
                        ps = psS.tile([P, 512], F32, tag="sc", name="sc")
                        nc.tensor.matmul(ps[:, lo:512], kslice,
                                         qt[:, :, lo:512],
                                         start=True, stop=True,
                                         perf_mode=DR, skip_group_check=True)
                        nc.scalar.activation(expT[:, j, lo:512], ps[:, lo:512],
                                             AF.Exp, bias=0.0, scale=SCALE)
                        ps2 = psS.tile([P, 512], F32, tag="sc", name="sc2")
                        nc.tensor.matmul(ps2[:], kslice, qt[:, :, 512:S],
                                         start=True, stop=True,
                                         perf_mode=DR, skip_group_check=True)
                        nc.scalar.activation(expT[:, j, 512:S], ps2[:],
                                             AF.Exp, bias=0.0, scale=SCALE)
                    else:
                        ps = psS.tile([P, 512], F32, tag="sc", name="sc")
                        nc.tensor.matmul(ps[:, lo - 512:512], kslice,
                                         qt[:, :, lo:S],
                                         start=True, stop=True,
                                         perf_mode=DR, skip_group_check=True)
                        nc.scalar.activation(expT[:, j, lo:S],
                                             ps[:, lo - 512:512],
                                             AF.Exp, bias=0.0, scale=SCALE)
                if js[-1] == SC - 1:
                    # strict-upper mask on the 8 diagonal blocks, one Pool op
                    if h % 2 == 1:
                        del pipe_qk[h // 2]
                    base = expT[:]
                    diag = bass.AP(tensor=base.tensor, offset=base.offset,
                                   ap=[base.ap[0], [S + P, SC], [1, P]])
                    mb = mask_rep[:]
                    mask3 = bass.AP(tensor=mb.tensor, offset=mb.offset,
                                    ap=[mb.ap[0], [P, SC], [1, P]])
                    nc.gpsimd.tensor_mul(diag, diag, mask3)

            # BMM2: fp8 DoubleRow over key-chunk pairs (K=256/instruction)
            def emit_bmm2_mm(h):
                expT = pipe_exp.pop(h)
                cds = []
                for n in range(2):
                    cd = psCD.tile([P, 512], F32, tag=f"cd{n}", name=f"cd{n}")
                    cds.append(cd)
                    # causal pairs for this column half
                    if n == 0:
                        pairs = [(0, 0), (2, 256)]
                    else:
                        pairs = [(0, 512), (2, 512), (4, 512), (6, 768)]
                    for pi, (j, c0) in enumerate(pairs):
                        nc.tensor.matmul(
                            cd[0:DK + 1, c0 - 512 * n:512],
                            vaug[:, j:j + 2, h, :],
                            expT[:, j:j + 2, c0:512 * (n + 1)],
                            start=(pi == 0), stop=(pi == len(pairs) - 1),
                            perf_mode=DR, skip_group_check=True)
                    rt = recp.tile([P, 512], F32R, tag="rec", name="rec")
                    nc.vector.tensor_copy(rt[DK:DK + 1, :], cd[DK:DK + 1, :])
                    cds.append(rt)
                pipe_cd[h] = cds

            def emit_bmm2_norm(h):
                cd0, rt0, cd1, rt1 = pipe_cd.pop(h)
                ch, off = h // 2, 64 * (h % 2)
                tmp = None
                if off:
                    tmp = recp.tile([64, S], F8, tag="tmp", name="tmp")
                for n, (cd, rt) in enumerate(((cd0, rt0), (cd1, rt1))):
                    # broadcast den across 64 partitions (PE), reciprocal to
                    # SBUF, normalize (single-PSUM-input multiply) with fp8
                    # quantize fused in.
                    pb = psS.tile([P, 512], F32, tag="sc", name="bc")
                    nc.tensor.matmul(pb[0:64, 0:512],
                                     ones1r[DK:DK + 1, :],
                                     rt[DK:DK + 1, :],
                                     start=True, stop=True,
                                     skip_group_check=True)
                    rb = recp.tile([64, 512], F32, tag="rb", name="rb")
                    nc.vector.reciprocal(rb[:], pb[0:64, 0:512])
                    dst = (concatT[0:64, ch, 512 * n:512 * (n + 1)]
                           if off == 0 else tmp[:, 512 * n:512 * (n + 1)])
                    nc.vector.tensor_mul(dst, cd[0:DK, :], rb[:])
                if off:
                    nc.sync.dma_start(concatT[64:P, ch, :], tmp[:])
                    # concat/8 for the Wo-residual correction GEMM (sws)
                    nc.gpsimd.tensor_scalar_mul(concat8[:, ch, :],
                                                concatT[:, ch, :], 0.125)

            for part in range(8):
                emit_qk_group(0, part // 4, part % 4)

            # ============ phase V: V projection (sbs, 3 DR GEMMs) ===========
            # interleaved with the first two heads' score matmuls so the
            # Scalar engine starts exp'ing while the PE runs V.
            def emit_v(m, nf):
                ps = psS.tile([P, 512], F32, tag="sc", name="v")[:, :]
                ncol = slice(512 * nf, 512 * (nf + 1))
                mcol = slice(P * m, P * (m + 1))
                for t in range(DP):
                    tp = slice(2 * t, 2 * t + 2)
                    nc.tensor.matmul(
                        ps, xT[:, tp, mcol], wvh[:, tp, ncol],
                        start=(t == 0), stop=False,
                        perf_mode=DR, skip_group_check=True)
                for t in range(DP):
                    tp = slice(2 * t, 2 * t + 2)
                    nc.tensor.matmul(
                        ps, xTl[:, tp, mcol], wvh[:, tp, ncol],
                        start=False, stop=False,
                        perf_mode=DR, skip_group_check=True)
                for t in range(DP):
                    tp = slice(2 * t, 2 * t + 2)
                    nc.tensor.matmul(
                        ps, xTh8[:, tp, mcol], wvl[:, tp, ncol],
                        start=False, stop=(t == DP - 1),
                        perf_mode=DR, skip_group_check=True)
                nc.scalar.activation(
                    vaug[:, m, 8 * nf:8 * (nf + 1), 0:DK],
                    ps.rearrange("p (h d) -> p h d", d=DK),
                    AF.Identity)

            emit_scores_js(0, [0, 1])
            for m in range(SC // 2):
                emit_v(m, 0)
                emit_v(m, 1)
            emit_scores_js(0, [2, 3])
            emit_qk_group(1, 0, 0)
            emit_scores_js(0, [4, 5, 6, 7])
            for m in range(SC // 2, SC):
                if m < SC - 1:
                    emit_qk_group(1, 0, m - 3)
                emit_v(m, 0)
                emit_v(m, 1)

            for h in range(2, H + 2):
                hh = h - 1
                g, u2 = hh // 2, hh % 2
                nxt = g + 1 < H // 2 and h <= H

                def qkc(ci):
                    c = 4 * u2 + ci
                    emit_qk_group(g + 1, c // 4, c % 4)

                if h <= H:
                    emit_scores_js(hh, [0, 1])
                if nxt:
                    qkc(0)
                if h >= 2:
                    emit_bmm2_mm(h - 2)
                if nxt:
                    qkc(1)
                if h <= H:
                    emit_scores_js(hh, [2, 3])
                if nxt:
                    qkc(2)
                if h >= 2:
                    emit_bmm2_norm(h - 2)
                if h <= H:
                    emit_scores_js(hh, [4, 5, 6, 7])
                if nxt:
                    qkc(3)

        attp_cm.__exit__(None, None, None)

        # ===== phase C/D: out-proj (sws, 2 DR GEMMs) + residual + LN1 + T ====
        with tc.tile_pool(name="xs", bufs=4) as xs, \
             tc.tile_pool(name="psC", bufs=4, space="PSUM") as psC, \
             tc.tile_pool(name="psT", bufs=3, space="PSUM") as psT:
            def emit_transpose(m):
                for cq in range(2):
                    pt = psT.tile([P, 4, P], F32, tag="t", name="t")
                    for ci in range(4):
                        c = 4 * cq + ci
                        nc.tensor.matmul(
                            pt[:, ci, :],
                            h1[:, m, P * c:P * (c + 1)],
                            ident[:], is_transpose=True,
                            start=True, stop=True, skip_group_check=True)
                    cols = slice(P * m, P * (m + 1))
                    cq4 = slice(4 * cq, 4 * (cq + 1))
                    hi = h1T_hi[:, cq4, cols]
                    nc.scalar.activation(hi, pt[:], AF.Identity)
                    nc.vector.scalar_tensor_tensor(
                        h1T_lo[:, cq4, cols], pt[:], 1.0, hi,
                        op0=OP.mult, op1=OP.subtract)
                    nc.gpsimd.tensor_scalar_mul(
                        h1T_h8[:, cq4, cols], hi, 0.125)

            for m in range(SC + 2):
                if m < SC:
                    xm = xs.tile([P, D], F32, tag="x", name=f"x{m}")
                    nc.sync.dma_start(xm[:], x_d[P * m:P * (m + 1), :])
                    stm = sm.tile([P, 2, 6], F32, tag="st", name="st")
                    mcol = slice(P * m, P * (m + 1))
                    for n in range(2):
                        pc = psC.tile([P, 512], F32, tag="c", name="c")
                        ncol = slice(512 * n, 512 * (n + 1))
                        for c in range(DP):
                            cp = slice(2 * c, 2 * c + 2)
                            nc.tensor.matmul(
                                pc[:], concatT[:, cp, mcol], woh[:, cp, ncol],
                                start=(c == 0), stop=False,
                                perf_mode=DR, skip_group_check=True)
                        for c in range(DP):
                            cp = slice(2 * c, 2 * c + 2)
                            nc.tensor.matmul(
                                pc[:], concat8[:, cp, mcol], wol[:, cp, ncol],
                                start=False, stop=(c == DP - 1),
                                perf_mode=DR, skip_group_check=True)
                        nc.vector.tensor_add(h1[:, m, ncol], pc[:], xm[:, ncol])
                        nc.vector.bn_stats(stm[:, n, :], h1[:, m, ncol])
                    ln_finish(stm, h1[:, m, :])
                if m >= 2:
                    emit_transpose(m - 2)

        midp_cm.__exit__(None, None, None)

        # ================= phase E: FFN1 (sbs, 3 DR GEMMs) =================
        ftp_cm = tc.tile_pool(name="ftp", bufs=1)
        ftp = ftp_cm.__enter__()
        fT_hi = ftp.tile([P, FC, S], F8)
        fT_lo = ftp.tile([P, FC, S], F8)
        fT_h8 = ftp.tile([P, FC, S], F8)
        with tc.tile_pool(name="w1s", bufs=6) as w1s, \
             tc.tile_pool(name="psF", bufs=2, space="PSUM") as psF:
            for c in range(FC):
                wth = w1s.tile([P, DC, P], F8, tag="w1h", name=f"w1h_{c}")
                wtl = w1s.tile([P, DC, P], F8, tag="w1l", name=f"w1l_{c}")
                nc.sync.dma_start(wth[:], w1h_d[c])
                nc.sync.dma_start(wtl[:], w1l_d[c])
                pf = psF.tile([P, 2, 512], F32, tag="f", name="f")
                for sh in range(2):
                    cols = slice(512 * sh, 512 * (sh + 1))
                    for t in range(DP):
                        tp = slice(2 * t, 2 * t + 2)
                        nc.tensor.matmul(
                            pf[:, sh, :], wth[:, tp, :], h1T_hi[:, tp, cols],
                            start=(t == 0), stop=False,
                            perf_mode=DR, skip_group_check=True)
                    for t in range(DP):
                        tp = slice(2 * t, 2 * t + 2)
                        nc.tensor.matmul(
                            pf[:, sh, :], wth[:, tp, :], h1T_lo[:, tp, cols],
                            start=False, stop=False,
                            perf_mode=DR, skip_group_check=True)
                    for t in range(DP):
                        tp = slice(2 * t, 2 * t + 2)
                        nc.tensor.matmul(
                            pf[:, sh, :], wtl[:, tp, :], h1T_h8[:, tp, cols],
                            start=False, stop=(t == DP - 1),
                            perf_mode=DR, skip_group_check=True)
                pff = pf[:].rearrange("p a b -> p (a b)")
                nc.scalar.activation(fT_hi[:, c, :], pff, AF.Relu)
                nc.vector.scalar_tensor_tensor(
                    fT_lo[:, c, :], pff, 0.0, fT_hi[:, c, :],
                    op0=OP.max, op1=OP.subtract)
                nc.gpsimd.tensor_scalar_mul(fT_h8[:, c, :], fT_hi[:, c, :],
                                            0.125)

        # ============ phase F: FFN2 (sbs, 3 DR GEMMs) + LN2 + out ===========
        w2h_r = w2h_d.rearrange("(c p) d -> p c d", p=P)
        w2l_r = w2l_d.rearrange("(c p) d -> p c d", p=P)
        with tc.tile_pool(name="w2s", bufs=6) as w2s, \
             tc.tile_pool(name="psG", bufs=1, space="PSUM") as psG:
            wts = {}

            def w2dma(i):
                half, c2 = divmod(i, FP2)
                wh = w2s.tile([P, 2, D], F8, tag="w2h", name=f"w2h_{half}_{c2}")
                wl = w2s.tile([P, 2, D], F8, tag="w2l", name=f"w2l_{half}_{c2}")
                cp = slice(2 * c2, 2 * c2 + 2)
                nc.sync.dma_start(wh[:], w2h_r[:, cp, :])
                nc.sync.dma_start(wl[:], w2l_r[:, cp, :])
                wts[i] = (wh, wl)

            def emit_out_chain(m, gtm):
                stm = sm.tile([P, 2, 6], F32, tag="st", name="st2")
                for n in range(2):
                    cols = slice(512 * n, 512 * (n + 1))
                    nc.vector.tensor_add(h1[:, m, cols], gtm[n][:],
                                         h1[:, m, cols])
                    nc.vector.bn_stats(stm[:, n, :], h1[:, m, cols])
                ln_finish(stm, h1[:, m, :])
                nc.sync.dma_start(out_d[P * m:P * (m + 1), :], h1[:, m, :])

            w2dma(0)
            w2dma(1)
            # each mi's K-loop lags the previous by one w2 chunk, so the
            # four accumulations finish staggered and their LN+store chains
            # overlap the remaining matmuls instead of draining at the end
            for half in range(2):
                gt = [[psG.tile([P, 512], F32, tag=f"g{mi}{n}",
                                name=f"g{half}{mi}{n}")
                       for n in range(2)] for mi in range(4)]
                for t in range(FP2 + 4):
                    i = half * FP2 + t
                    if t < FP2 and i + 2 < 2 * FP2:
                        w2dma(i + 2)
                    for mi in range(4):
                        c2 = t - mi
                        if not (0 <= c2 < FP2):
                            continue
                        wh, wl = wts[half * FP2 + c2]
                        cp = slice(2 * c2, 2 * c2 + 2)
                        m = 4 * half + mi
                        mcol = slice(P * m, P * (m + 1))
                        for n in range(2):
                            ncol = slice(512 * n, 512 * (n + 1))
                            nc.tensor.matmul(
                                gt[mi][n][:], fT_hi[:, cp, mcol],
                                wh[:, :, ncol],
                                start=(c2 == 0), stop=False,
                                perf_mode=DR, skip_group_check=True)
                            nc.tensor.matmul(
                                gt[mi][n][:], fT_lo[:, cp, mcol],
                                wh[:, :, ncol],
                                start=False, stop=False,
                                perf_mode=DR, skip_group_check=True)
                            nc.tensor.matmul(
                                gt[mi][n][:], fT_h8[:, cp, mcol],
                                wl[:, :, ncol],
                                start=False, stop=(c2 == FP2 - 1),
                                perf_mode=DR, skip_group_check=True)
                        if c2 == FP2 - 1:
                            emit_out_chain(m, gt[mi])
                    if 0 <= t - 3 < FP2:
                        wts.pop(half * FP2 + t - 3)
        ftp_cm.__exit__(None, None, None)

    nc.compile()
    return nc


_cached = None


def _get_prog():
    global _cached
    if _cached is None:
        _cached = _build()
    return _cached


def _ln_np(v, g, b):
    mu = v.mean(-1, keepdims=True)
    var = v.var(-1, keepdims=True)
    return (v - mu) / np.sqrt(var + EPS) * g + b


def _host_row0(inputs):
    """Exact f32 recompute of output row 0 (attention row 0 is zero-padded)."""
    x0 = np.asarray(inputs["x"], np.float32)[:, 0, :]          # [B, D]
    h1 = _ln_np(x0 + inputs["bo"][None, :], inputs["g1"], inputs["beta1"])
    f1 = np.maximum(h1 @ inputs["W1"] + inputs["b1"], 0.0)
    ffn = f1 @ inputs["W2"] + inputs["b2"]
    return _ln_np(h1 + ffn, inputs["g3"], inputs["beta3"])


def _fallback_np(inputs):
    x = np.asarray(inputs["x"], np.float32)
    q = x @ inputs["Wq"] + inputs["bq"]
    k = x @ inputs["Wk"] + inputs["bk"]
    v = x @ inputs["Wv"] + inputs["bv"]

    def heads(t):
        return t.reshape(B, S, H, DK).transpose(0, 2, 1, 3)
    q, k, v = heads(q), heads(k), heads(v)
    sc = np.einsum('bhqd,bhkd->bhqk', q, k).astype(np.float32) / 8.0
    i = np.arange(S)[:, None]
    j = np.arange(S)[None, :]
    sc = np.where((j < i)[None, None], sc, -1e32)
    sc -= sc.max(-1, keepdims=True)
    e = np.exp(sc)
    att = e / e.sum(-1, keepdims=True)
    att = att * (i > 0)[None, None]
    ctx = np.einsum('bhqk,bhkd->bhqd', att, v)
    concat = ctx.transpose(0, 2, 1, 3).reshape(B, S, D)
    h1 = _ln_np(x + concat @ inputs["Wo"] + inputs["bo"],
                inputs["g1"], inputs["beta1"])
    f1 = np.maximum(h1 @ inputs["W1"] + inputs["b1"], 0.0)
    out = _ln_np(h1 + f1 @ inputs["W2"] + inputs["b2"],
                 inputs["g3"], inputs["beta3"])
    return out.astype(np.float32)


def _fast_path_ok(inputs):
    z = lambda n: not np.any(np.asarray(inputs[n]))
    o = lambda n: np.all(np.asarray(inputs[n]) == 1.0)
    return (z("bq") and z("bk") and z("bv") and z("bo") and z("b1")
            and z("b2") and z("beta1") and z("beta3") and o("g1") and o("g3"))


def _split8(w):
    """fp8 hi + scaled lo decomposition of a weight matrix."""
    w = np.asarray(w, np.float32)
    hi = w.astype(NP_F8)
    lo8 = (8.0 * (w - hi.astype(np.float32))).astype(NP_F8)
    return hi, lo8


def kernel(**inputs):
    x = np.asarray(inputs["x"], dtype=np.float32)
    assert x.shape == (B, S, D)
    if not _fast_path_ok(inputs):
        return _fallback_np(inputs)

    f8 = lambda a: np.ascontiguousarray(np.asarray(a)).astype(NP_F8)
    # permute Wq/Wk columns so each 2-head group's 64 projection outputs
    # per d-half (heads 2g+u2, d-half i) are contiguous: col
    # 128g+64i+32u2+d32 maps to original head 2g+u2, d = 32i + d32.
    col = np.arange(D)
    g_, r_ = np.divmod(col, 128)
    i_, r2_ = np.divmod(r_, 64)
    u_, d_ = np.divmod(r2_, 32)
    perm = 64 * (2 * g_ + u_) + 32 * i_ + d_
    wvh, wvl = _split8(inputs["Wv"])
    woh, wol = _split8(inputs["Wo"])
    w1 = np.asarray(inputs["W1"], np.float32)
    w1t = np.ascontiguousarray(
        w1.reshape(DC, P, FC, P).transpose(2, 1, 0, 3))
    w1h = w1t.astype(NP_F8)
    w1l = (8.0 * (w1t - w1h.astype(np.float32))).astype(NP_F8)
    w2h, w2l = _split8(inputs["W2"])
    common = dict(
        wq8=f8(np.asarray(inputs["Wq"])[:, perm]),
        wk8=f8(np.asarray(inputs["Wk"])[:, perm]),
        wvh=wvh, wvl=wvl, woh=woh, wol=wol,
        w1h=w1h, w1l=w1l, w2h=w2h, w2l=w2l,
        mask8=np.triu(np.ones((P, P), np.float32), k=1).astype(NP_F8),
        ident=np.eye(P, dtype=np.float32),
        onesf=np.ones((P, 64), np.float32),
    )
    in_maps = []
    for i in range(B):
        xi = np.ascontiguousarray(x[i])
        xiT = np.ascontiguousarray(xi.T)
        xhi = xiT.astype(NP_F8)
        xlo = (xiT - xhi.astype(np.float32)).astype(NP_F8)
        xh8 = (xhi.astype(np.float32) / 8.0).astype(NP_F8)
        in_maps.append(dict(common, x32=xi, xT8=xhi, xTl8=xlo, xTh8=xh8))
    nc = _get_prog()
    res = run_bass_kernel_spmd(nc, in_maps, list(range(B)))
    out = np.stack([res.results[i]["out"] for i in range(B)], axis=0)
    out[:, 0, :] = _host_row0(inputs)
    return out


# revision 39
# speedup vs baseline: 1.1738x; 1.0025x over previous
"""Trainium2 Bass kernel for nn_BasicBlock (dense transformer block).

Data-parallel over batch: B=8 batch elements, one per NeuronCore, no
collectives. fp8e4m3-DoubleRow on every GEMM that tolerates it, with
scaled hi/lo error-correction splits where single fp8 would break the
2e-2 budget.

Numerics scheme (validated in numpy prototype, proto3.py):
  - A GEMM a@W in "sbs" (split-both-scaled) form runs three fp8-DR GEMMs
    accumulating into one PSUM: a_hi@W_hi + a_lo@W_hi + (a_hi/8)@(8*W_lo).
    The x8/ :8 scaling keeps the W-residual above e4m3's subnormal floor
    (W ~ 0.02 scale would otherwise flush to zero). Residual error ~0.1%.
  - FFN1, FFN2, V-proj: sbs (exact-ish). Out-proj: sws (W-corrected,
    concat single-quant). Scores Q/K bf16 storage (stage A), BMM2 in fp8
    DR with paired key-chunks (K=256/instruction).
  - Predicted rel err ~ 8e-3 vs the 2e-2 budget.

Per-core layout (S=1024, D=1024, H=16, d_k=64, d_ff=4096):
  - Q/K projections fp8 DoubleRow as before; scoresT bf16 per
    (head, key-chunk), causal columns only; exp on ScalarE writes fp8
    expT; strict lower-triangle mask via diagonal-strided DVE multiply;
    BMM2 runs DR over key-chunk PAIRS (the pre-zeroed odd-j gap regions
    make the paired reads causal-safe); ones-augmented V accumulates the
    softmax denominator; normalize fused into the fp8 quantize multiply.
  - Attention row 0 (zero-pad row) produces NaNs on device; host
    recomputes output row 0 exactly in numpy.
  - Fast path assumes the reference's structural constants (biases 0,
    gammas 1, betas 0); other inputs fall back to numpy.
"""

import numpy as np
import ml_dtypes
import concourse.bass as bass
import concourse.tile as tile
from concourse import bacc, mybir
from concourse.bass_utils import run_bass_kernel_spmd

F32 = mybir.dt.float32
F32R = mybir.dt.float32r
F8 = mybir.dt.float8e4
BF16 = mybir.dt.bfloat16
AF = mybir.ActivationFunctionType
OP = mybir.AluOpType
DR = mybir.MatmulPerfMode.DoubleRow

NP_F8 = ml_dtypes.float8_e4m3
NP_BF16 = ml_dtypes.bfloat16

B, S, D, H, DK, DFF = 8, 1024, 1024, 16, 64, 4096
P = 128
DC = D // P       # 8 chunks of d_model
DP = DC // 2      # 4 K-pairs of d_model (DR)
FC = DFF // P     # 32 chunks of d_ff
FP2 = FC // 2     # 16 K-pairs of d_ff (DR)
SC = S // P       # 8 chunks of sequence
EPS = 1e-5
SCALE = 0.125     # 1/sqrt(DK)


def _build():
    nc = bacc.Bacc("TRN2", target_bir_lowering=False, debug=False, num_devices=B)

    # per-batch inputs
    xT_d = nc.dram_tensor("xT8", [D, S], F8, kind="ExternalInput").ap()
    xTl_d = nc.dram_tensor("xTl8", [D, S], F8, kind="ExternalInput").ap()
    xTh8_d = nc.dram_tensor("xTh8", [D, S], F8, kind="ExternalInput").ap()
    x_d = nc.dram_tensor("x32", [S, D], F32, kind="ExternalInput").ap()
    # common weights
    wq_d = nc.dram_tensor("wq8", [D, D], F8, kind="ExternalInput").ap()
    wk_d = nc.dram_tensor("wk8", [D, D], F8, kind="ExternalInput").ap()
    wvh_d = nc.dram_tensor("wvh", [D, D], F8, kind="ExternalInput").ap()
    wvl_d = nc.dram_tensor("wvl", [D, D], F8, kind="ExternalInput").ap()
    woh_d = nc.dram_tensor("woh", [D, D], F8, kind="ExternalInput").ap()
    wol_d = nc.dram_tensor("wol", [D, D], F8, kind="ExternalInput").ap()
    # host pre-arranged: w1p[c, p, t, f] = W1[128t+p, 128c+f]
    w1h_d = nc.dram_tensor("w1h", [FC, P, DC, P], F8, kind="ExternalInput").ap()
    w1l_d = nc.dram_tensor("w1l", [FC, P, DC, P], F8, kind="ExternalInput").ap()
    w2h_d = nc.dram_tensor("w2h", [DFF, D], F8, kind="ExternalInput").ap()
    w2l_d = nc.dram_tensor("w2l", [DFF, D], F8, kind="ExternalInput").ap()
    mask_d = nc.dram_tensor("mask8", [P, P], F8, kind="ExternalInput").ap()
    id_d = nc.dram_tensor("ident", [P, P], F32, kind="ExternalInput").ap()
    ones_d = nc.dram_tensor("onesf", [P, 64], F32, kind="ExternalInput").ap()
    out_d = nc.dram_tensor("out", [S, D], F32, kind="ExternalOutput").ap()

    with tile.TileContext(nc) as tc:
      with tc.tile_pool(name="singles", bufs=1) as sg, \
           tc.tile_pool(name="sm", bufs=6) as sm:
        mask_rep = sg.tile([P, SC, P], F8)
        ident = sg.tile([P, P], F32)
        ones1r = sg.tile([P, 64], F32R)
        eps_t = sg.tile([P, 1], F32)
        h1 = sg.tile([P, SC, D], F32)
        h1T_hi = sg.tile([P, DC, S], F8)
        h1T_lo = sg.tile([P, DC, S], F8)
        h1T_h8 = sg.tile([P, DC, S], F8)

        nc.vector.memset(eps_t[:], EPS)

        # pool spanning V-proj .. out-proj/LN1
        midp_cm = tc.tile_pool(name="midp", bufs=1)
        mp_ = midp_cm.__enter__()
        concatT = mp_.tile([P, DC, S], F8)
        concat8 = mp_.tile([P, DC, S], F8)
        woh = mp_.tile([P, DC, D], F8)
        wol = mp_.tile([P, DC, D], F8)

        attp_cm = tc.tile_pool(name="attp", bufs=1)
        ap_ = attp_cm.__enter__()
        xT = ap_.tile([P, DC, S], F8)
        xTl = ap_.tile([P, DC, S], F8)
        xTh8 = ap_.tile([P, DC, S], F8)
        wq = ap_.tile([P, DC, D], F8)
        wk = ap_.tile([P, DC, D], F8)
        wvh = ap_.tile([P, DC, D], F8)
        wvl = ap_.tile([P, DC, D], F8)
        vaug = ap_.tile([P, SC, H, DK + 1], F8)

        # load order tuned for startup: x first, then the weights in the
        # order the first phases consume them (QK group0, V hi/lo/h8, ...)
        xT_r = xT_d.rearrange("(c p) s -> p c s", p=P)
        wq_r = wq_d.rearrange("(c p) d -> p c d", p=P)
        wk_r = wk_d.rearrange("(c p) d -> p c d", p=P)
        wvh_r = wvh_d.rearrange("(c p) d -> p c d", p=P)
        nc.sync.dma_start(xT[:, :, 0:256], xT_r[:, :, 0:256])
        nc.sync.dma_start(wq[:, :, 0:128], wq_r[:, :, 0:128])
        nc.sync.dma_start(wk[:, :, 0:128], wk_r[:, :, 0:128])
        nc.sync.dma_start(xT[:, :, 256:512], xT_r[:, :, 256:512])
        nc.sync.dma_start(xT[:, :, 512:768], xT_r[:, :, 512:768])
        nc.sync.dma_start(xT[:, :, 768:S], xT_r[:, :, 768:S])
        nc.sync.dma_start(wvh[:, 0:4, :], wvh_r[:, 0:4, :])
        nc.sync.dma_start(wvh[:, 4:8, :], wvh_r[:, 4:8, :])
        nc.sync.dma_start(ones1r[:], ones_d.bitcast(F32R))
        # mask replicated 8x along a middle dim for the diagonal-AP multiply
        mask_bc = bass.AP(tensor=mask_d.tensor, offset=mask_d.offset,
                          ap=[[P, P], [0, SC], [1, P]])
        nc.sync.dma_start(mask_rep[:], mask_bc)
        nc.sync.dma_start(ident[:], id_d)
        nc.sync.dma_start(xTl[:], xTl_d.rearrange("(c p) s -> p c s", p=P))
        nc.sync.dma_start(xTh8[:], xTh8_d.rearrange("(c p) s -> p c s", p=P))
        nc.sync.dma_start(wvl[:], wvl_d.rearrange("(c p) d -> p c d", p=P))
        nc.sync.dma_start(wq[:, :, 128:D], wq_r[:, :, 128:D])
        nc.sync.dma_start(wk[:, :, 128:D], wk_r[:, :, 128:D])
        nc.sync.dma_start(woh[:], woh_d.rearrange("(c p) d -> p c d", p=P))
        nc.sync.dma_start(wol[:], wol_d.rearrange("(c p) d -> p c d", p=P))
        nc.vector.memset(vaug[:, :, :, DK:DK + 1], 1.0)

        def ln_finish(stm, ap_full):
            # stm [P, 2, 6] bn_stats pairs; ap_full [P, 1024] in SBUF
            mv = sm.tile([P, 2], F32, tag="mv", name="mv")
            rs = sm.tile([P, 1], F32, tag="rs", name="rs")
            nb = sm.tile([P, 1], F32, tag="nb", name="nb")
            nc.vector.bn_aggr(mv[:], stm[:])
            nc.scalar.activation(rs[:], mv[:, 1:2], AF.Sqrt,
                                 bias=eps_t[:], scale=1.0)
            nc.vector.reciprocal(rs[:], rs[:])
            nc.vector.tensor_scalar(nb[:], mv[:, 0:1], rs[:], -1.0,
                                    op0=OP.mult, op1=OP.mult)
            nc.scalar.activation(ap_full, ap_full, AF.Identity,
                                 bias=nb[:], scale=rs[:])

        # ================= attention =================
        with tc.tile_pool(name="qkp", bufs=3) as qkp, \
             tc.tile_pool(name="expp", bufs=3) as expp, \
             tc.tile_pool(name="recp", bufs=1) as recp, \
             tc.tile_pool(name="psQK", bufs=2, space="PSUM") as psQK, \
             tc.tile_pool(name="psS", bufs=4, space="PSUM") as psS, \
             tc.tile_pool(name="psCD", bufs=1, space="PSUM") as psCD:

            # pre-zero the odd-j gap regions of the expT buffers:
            # (p, j=2jp+1, c in [256jp, 256jp+128)) is read by paired BMM2
            # but never written by exp.
            for i in range(3):
                e = expp.tile([P, SC, S], F8, tag="expT", name=f"expT{i}")
                base = e[:]
                gap = bass.AP(tensor=base.tensor, offset=base.offset + S,
                              ap=[base.ap[0], [2 * S + 256, 4], [1, P]])
                nc.vector.memset(gap, 0.0)

            pipe_qk = {}
            pipe_exp = {}
            pipe_cd = {}

            # Q/K projections: 2 heads packed per group g; q/k stored as
            # [64, 2, S] fp8 where partition 32*u2 + d32, plane i hold head
            # 2g+u2, d = d32 + 32*i (u2 head-in-group).  Wq/Wk are
            # host-permuted so a group's 64 output columns per d-half are
            # contiguous.  This layout lets the d_k=64-contraction score
            # matmuls run fp8 DoubleRow as [32, 2, *] APs (AP partition base
            # limited to 0/32/64, so at most 2 heads share a tile).
            def emit_qk_group(g, qk, quarter):
                if qk == 0 and quarter == 0:
                    qt = qkp.tile([64, 2, S], F8, tag="qT", name=f"qT{g}")
                    kt = qkp.tile([64, 2, S], F8, tag="kT", name=f"kT{g}")
                    pipe_qk[g] = (qt, kt)
                qt, kt = pipe_qk[g]
                dst = qt if qk == 0 else kt
                w = wq if qk == 0 else wk
                pq = psQK.tile([64, 2, 256], F32, tag="pq", name="pq")
                scol = 256 * quarter
                for i in range(2):
                    wcol = 128 * g + 64 * i
                    for t in range(4):
                        nc.tensor.matmul(
                            pq[:, i, :],
                            w[:, 2 * t:2 * t + 2, wcol:wcol + 64],
                            xT[:, 2 * t:2 * t + 2, scol:scol + 256],
                            start=(t == 0), stop=(t == 3),
                            perf_mode=DR, skip_group_check=True)
                dcols = dst[:, :, scol:scol + 256]
                nc.vector.tensor_copy(dcols, pq[:])

            def emit_scores_js(h, js):
                if js[0] == 0:
                    pipe_exp[h] = expp.tile([P, SC, S], F8, tag="expT",
                                            name=f"expT_h{h}")
                qt0, kt0 = pipe_qk[h // 2]
                u = 32 * (h % 2)
                qt = qt0[u:u + 32, :, :]
                expT = pipe_exp[h]
                for j in js:
                    lo = P * j
                    kslice = kt0[u:u + 32, :, lo:lo + P]
                    if lo < 512:
                        ps = psS.tile([P, 512], F32, tag="sc", name="sc")
                        nc.tensor.matmul(ps[:, lo:512], kslice,
                                         qt[:, :, lo:512],
                                         start=True, stop=True,
                                         perf_mode=DR, skip_group_check=True)
                        nc.scalar.activation(expT[:, j, lo:512], ps[:, lo:512],
                                             AF.Exp, bias=0.0, scale=SCALE)
                        ps2 = psS.tile([P, 512], F32, tag="sc", name="sc2")
                        nc.tensor.matmul(ps2[:], kslice, qt[:, :, 512:S],
                                         start=True, stop=True,
                                         perf_mode=DR, skip_group_check=True)
                        nc.scalar.activation(expT[:, j, 512:S], ps2[:],
                                             AF.Exp, bias=0.0, scale=SCALE)
                    else:
                        ps = psS.tile([P, 512], F32, tag="sc", name="sc")
                        nc.tensor.matmul(ps[:, lo - 512:512], kslice,
                                         qt[:, :, lo:S],
                                         start=True, stop=True,
                                         perf_mode=DR, skip_group_check=True)
                        nc.scalar.activation(expT[:, j, lo:S],
                                             ps[:, lo - 512:512],
                                             AF.Exp, bias=0.0, scale=SCALE)
                if js[-1] == SC - 1:
                    # strict-upper mask on the 8 diagonal blocks, one Pool op
                    if h % 2 == 1:
                        del pipe_qk[h // 2]
                    base = expT[:]
                    diag = bass.AP(tensor=base.tensor, offset=base.offset,
                                   ap=[base.ap[0], [S + P, SC], [1, P]])
                    mb = mask_rep[:]
                    mask3 = bass.AP(tensor=mb.tensor, offset=mb.offset,
                                    ap=[mb.ap[0], [P, SC], [1, P]])
                    nc.gpsimd.tensor_mul(diag, diag, mask3)

            # BMM2: fp8 DoubleRow over key-chunk pairs (K=256/instruction)
            def emit_bmm2_mm(h):
                expT = pipe_exp.pop(h)
                cds = []
                for n in range(2):
                    cd = psCD.tile([P, 512], F32, tag=f"cd{n}", name=f"cd{n}")
                    cds.append(cd)
                    # causal pairs for this column half
                    if n == 0:
                        pairs = [(0, 0), (2, 256)]
                    else:
                        pairs = [(0, 512), (2, 512), (4, 512), (6, 768)]
                    for pi, (j, c0) in enumerate(pairs):
                        nc.tensor.matmul(
                            cd[0:DK + 1, c0 - 512 * n:512],
                            vaug[:, j:j + 2, h, :],
                            expT[:, j:j + 2, c0:512 * (n + 1)],
                            start=(pi == 0), stop=(pi == len(pairs) - 1),
                            perf_mode=DR, skip_group_check=True)
                    rt = recp.tile([P, 512], F32R, tag="rec", name="rec")
                    nc.vector.tensor_copy(rt[DK:DK + 1, :], cd[DK:DK + 1, :])
                    cds.append(rt)
                pipe_cd[h] = cds

            def emit_bmm2_norm(h):
                cd0, rt0, cd1, rt1 = pipe_cd.pop(h)
                ch, off = h // 2, 64 * (h % 2)
                tmp = None
                if off:
                    tmp = recp.tile([64, S], F8, tag="tmp", name="tmp")
                for n, (cd, rt) in enumerate(((cd0, rt0), (cd1, rt1))):
                    # broadcast den across 64 partitions (PE), reciprocal to
                    # SBUF, normalize (single-PSUM-input multiply) with fp8
                    # quantize fused in.
                    pb = psS.tile([P, 512], F32, tag="sc", name="bc")
                    nc.tensor.matmul(pb[0:64, 0:512],
                                     ones1r[DK:DK + 1, :],
                                     rt[DK:DK + 1, :],
                                     start=True, stop=True,
                                     skip_group_check=True)
                    rb = recp.tile([64, 512], F32, tag="rb", name="rb")
                    nc.vector.reciprocal(rb[:], pb[0:64, 0:512])
                    dst = (concatT[0:64, ch, 512 * n:512 * (n + 1)]
                           if off == 0 else tmp[:, 512 * n:512 * (n + 1)])
                    nc.vector.tensor_mul(dst, cd[0:DK, :], rb[:])
                if off:
                    nc.sync.dma_start(concatT[64:P, ch, :], tmp[:])
                    # concat/8 for the Wo-residual correction GEMM (sws)
                    nc.gpsimd.tensor_scalar_mul(concat8[:, ch, :],
                                                concatT[:, ch, :], 0.125)

            for part in range(8):
                emit_qk_group(0, part // 4, part % 4)

            # ============ phase V: V projection (sbs, 3 DR GEMMs) ===========
            # interleaved with the first two heads' score matmuls so the
            # Scalar engine starts exp'ing while the PE runs V.
            def emit_v(m, nf):
                ps = psS.tile([P, 512], F32, tag="sc", name="v")[:, :]
                ncol = slice(512 * nf, 512 * (nf + 1))
                mcol = slice(P * m, P * (m + 1))
                for t in range(DP):
                    tp = slice(2 * t, 2 * t + 2)
                    nc.tensor.matmul(
                        ps, xT[:, tp, mcol], wvh[:, tp, ncol],
                        start=(t == 0), stop=False,
                        perf_mode=DR, skip_group_check=True)
                for t in range(DP):
                    tp = slice(2 * t, 2 * t + 2)
                    nc.tensor.matmul(
                        ps, xTl[:, tp, mcol], wvh[:, tp, ncol],
                        start=False, stop=False,
                        perf_mode=DR, skip_group_check=True)
                for t in range(DP):
                    tp = slice(2 * t, 2 * t + 2)
                    nc.tensor.matmul(
                        ps, xTh8[:, tp, mcol], wvl[:, tp, ncol],
                        start=False, stop=(t == DP - 1),
                        perf_mode=DR, skip_group_check=True)
                nc.scalar.activation(
                    vaug[:, m, 8 * nf:8 * (nf + 1), 0:DK],
                    ps.rearrange("p (h d) -> p h d", d=DK),
                    AF.Identity)

            emit_scores_js(0, [0, 1])
            for m in range(SC // 2):
                emit_v(m, 0)
                emit_v(m, 1)
            emit_scores_js(0, [2, 3])
            emit_qk_group(1, 0, 0)
            emit_scores_js(0, [4, 5, 6, 7])
            for m in range(SC // 2, SC):
                if m < SC - 1:
                    emit_qk_group(1, 0, m - 3)
                emit_v(m, 0)
                emit_v(m, 1)

            for h in range(2, H + 2):
                hh = h - 1
                g, u2 = hh // 2, hh % 2
                nxt = hh <= H - 1 and g + 1 < H // 2

                def qkc(ci):
                    c = 4 * u2 + ci
                    emit_qk_group(g + 1, c // 4, c % 4)

                if h <= H:
                    emit_scores_js(hh, [0, 1])
                if nxt:
                    qkc(0)
                emit_bmm2_mm(h - 2)
                if nxt:
                    qkc(1)
                if h <= H:
                    emit_scores_js(hh, [2, 3])
                if nxt:
                    qkc(2)
                emit_bmm2_norm(h - 2)
                if h <= H:
                    emit_scores_js(hh, [4, 5, 6, 7])
                if nxt:
                    qkc(3)

        attp_cm.__exit__(None, None, None)

        # ===== phase C/D: out-proj (sws, 2 DR GEMMs) + residual + LN1 + T ====
        with tc.tile_pool(name="xs", bufs=4) as xs, \
             tc.tile_pool(name="psC", bufs=4, space="PSUM") as psC, \
             tc.tile_pool(name="psT", bufs=3, space="PSUM") as psT:
            def emit_transpose(m):
                for cq in range(2):
                    pt = psT.tile([P, 4, P], F32, tag="t", name="t")
                    for ci in range(4):
                        c = 4 * cq + ci
                        nc.tensor.matmul(
                            pt[:, ci, :],
                            h1[:, m, P * c:P * (c + 1)],
                            ident[:], is_transpose=True,
                            start=True, stop=True, skip_group_check=True)
                    cols = slice(P * m, P * (m + 1))
                    cq4 = slice(4 * cq, 4 * (cq + 1))
                    hi = h1T_hi[:, cq4, cols]
                    nc.scalar.activation(hi, pt[:], AF.Identity)
                    nc.vector.scalar_tensor_tensor(
                        h1T_lo[:, cq4, cols], pt[:], 1.0, hi,
                        op0=OP.mult, op1=OP.subtract)
                    nc.gpsimd.tensor_scalar_mul(
                        h1T_h8[:, cq4, cols], hi, 0.125)

            for m in range(SC + 2):
                if m < SC:
                    xm = xs.tile([P, D], F32, tag="x", name=f"x{m}")
                    nc.sync.dma_start(xm[:], x_d[P * m:P * (m + 1), :])
                    stm = sm.tile([P, 2, 6], F32, tag="st", name="st")
                    mcol = slice(P * m, P * (m + 1))
                    for n in range(2):
                        pc = psC.tile([P, 512], F32, tag="c", name="c")
                        ncol = slice(512 * n, 512 * (n + 1))
                        for c in range(DP):
                            cp = slice(2 * c, 2 * c + 2)
                            nc.tensor.matmul(
                                pc[:], concatT[:, cp, mcol], woh[:, cp, ncol],
                                start=(c == 0), stop=False,
                                perf_mode=DR, skip_group_check=True)
                        for c in range(DP):
                            cp = slice(2 * c, 2 * c + 2)
                            nc.tensor.matmul(
                                pc[:], concat8[:, cp, mcol], wol[:, cp, ncol],
                                start=False, stop=(c == DP - 1),
                                perf_mode=DR, skip_group_check=True)
                        nc.vector.tensor_add(h1[:, m, ncol], pc[:], xm[:, ncol])
                        nc.vector.bn_stats(stm[:, n, :], h1[:, m, ncol])
                    ln_finish(stm, h1[:, m, :])
                if m >= 2:
                    emit_transpose(m - 2)

        midp_cm.__exit__(None, None, None)

        # ================= phase E: FFN1 (sbs, 3 DR GEMMs) =================
        ftp_cm = tc.tile_pool(name="ftp", bufs=1)
        ftp = ftp_cm.__enter__()
        fT_hi = ftp.tile([P, FC, S], F8)
        fT_lo = ftp.tile([P, FC, S], F8)
        fT_h8 = ftp.tile([P, FC, S], F8)
        with tc.tile_pool(name="w1s", bufs=6) as w1s, \
             tc.tile_pool(name="psF", bufs=2, space="PSUM") as psF:
            for c in range(FC):
                wth = w1s.tile([P, DC, P], F8, tag="w1h", name=f"w1h_{c}")
                wtl = w1s.tile([P, DC, P], F8, tag="w1l", name=f"w1l_{c}")
                nc.sync.dma_start(wth[:], w1h_d[c])
                nc.sync.dma_start(wtl[:], w1l_d[c])
                pf = psF.tile([P, 2, 512], F32, tag="f", name="f")
                for sh in range(2):
                    cols = slice(512 * sh, 512 * (sh + 1))
                    for t in range(DP):
                        tp = slice(2 * t, 2 * t + 2)
                        nc.tensor.matmul(
                            pf[:, sh, :], wth[:, tp, :], h1T_hi[:, tp, cols],
                            start=(t == 0), stop=False,
                            perf_mode=DR, skip_group_check=True)
                    for t in range(DP):
                        tp = slice(2 * t, 2 * t + 2)
                        nc.tensor.matmul(
                            pf[:, sh, :], wth[:, tp, :], h1T_lo[:, tp, cols],
                            start=False, stop=False,
                            perf_mode=DR, skip_group_check=True)
                    for t in range(DP):
                        tp = slice(2 * t, 2 * t + 2)
                        nc.tensor.matmul(
                            pf[:, sh, :], wtl[:, tp, :], h1T_h8[:, tp, cols],
                            start=False, stop=(t == DP - 1),
                            perf_mode=DR, skip_group_check=True)
                pff = pf[:].rearrange("p a b -> p (a b)")
                nc.scalar.activation(fT_hi[:, c, :], pff, AF.Relu)
                nc.vector.scalar_tensor_tensor(
                    fT_lo[:, c, :], pff, 0.0, fT_hi[:, c, :],
                    op0=OP.max, op1=OP.subtract)
                nc.gpsimd.tensor_scalar_mul(fT_h8[:, c, :], fT_hi[:, c, :],
                                            0.125)

        # ============ phase F: FFN2 (sbs, 3 DR GEMMs) + LN2 + out ===========
        w2h_r = w2h_d.rearrange("(c p) d -> p c d", p=P)
        w2l_r = w2l_d.rearrange("(c p) d -> p c d", p=P)
        with tc.tile_pool(name="w2s", bufs=6) as w2s, \
             tc.tile_pool(name="psG", bufs=1, space="PSUM") as psG:
            wts = {}

            def w2dma(i):
                half, c2 = divmod(i, FP2)
                wh = w2s.tile([P, 2, D], F8, tag="w2h", name=f"w2h_{half}_{c2}")
                wl = w2s.tile([P, 2, D], F8, tag="w2l", name=f"w2l_{half}_{c2}")
                cp = slice(2 * c2, 2 * c2 + 2)
                nc.sync.dma_start(wh[:], w2h_r[:, cp, :])
                nc.sync.dma_start(wl[:], w2l_r[:, cp, :])
                wts[i] = (wh, wl)

            def emit_out_chain(m, gtm):
                stm = sm.tile([P, 2, 6], F32, tag="st", name="st2")
                for n in range(2):
                    cols = slice(512 * n, 512 * (n + 1))
                    nc.vector.tensor_add(h1[:, m, cols], gtm[n][:],
                                         h1[:, m, cols])
                    nc.vector.bn_stats(stm[:, n, :], h1[:, m, cols])
                ln_finish(stm, h1[:, m, :])
                nc.sync.dma_start(out_d[P * m:P * (m + 1), :], h1[:, m, :])

            w2dma(0)
            w2dma(1)
            # each mi's K-loop lags the previous by one w2 chunk, so the
            # four accumulations finish staggered and their LN+store chains
            # overlap the remaining matmuls instead of draining at the end
            for half in range(2):
                gt = [[psG.tile([P, 512], F32, tag=f"g{mi}{n}",
                                name=f"g{half}{mi}{n}")
                       for n in range(2)] for mi in range(4)]
                for t in range(FP2 + 4):
                    i = half * FP2 + t
                    if t < FP2 and i + 2 < 2 * FP2:
                        w2dma(i + 2)
                    for mi in range(4):
                        c2 = t - mi
                        if not (0 <= c2 < FP2):
                            continue
                        wh, wl = wts[half * FP2 + c2]
                        cp = slice(2 * c2, 2 * c2 + 2)
                        m = 4 * half + mi
                        mcol = slice(P * m, P * (m + 1))
                        for n in range(2):
                            ncol = slice(512 * n, 512 * (n + 1))
                            nc.tensor.matmul(
                                gt[mi][n][:], fT_hi[:, cp, mcol],
                                wh[:, :, ncol],
                                start=(c2 == 0), stop=False,
                                perf_mode=DR, skip_group_check=True)
                            nc.tensor.matmul(
                                gt[mi][n][:], fT_lo[:, cp, mcol],
                                wh[:, :, ncol],
                                start=False, stop=False,
                                perf_mode=DR, skip_group_check=True)
                            nc.tensor.matmul(
                                gt[mi][n][:], fT_h8[:, cp, mcol],
                                wl[:, :, ncol],
                                start=False, stop=(c2 == FP2 - 1),
                                perf_mode=DR, skip_group_check=True)
                        if c2 == FP2 - 1:
                            emit_out_chain(m, gt[mi])
                    if 0 <= t - 3 < FP2:
                        wts.pop(half * FP2 + t - 3)
        ftp_cm.__exit__(None, None, None)

    nc.compile()
    return nc


_cached = None


def _get_prog():
    global _cached
    if _cached is None:
        _cached = _build()
    return _cached


def _ln_np(v, g, b):
    mu = v.mean(-1, keepdims=True)
    var = v.var(-1, keepdims=True)
    return (v - mu) / np.sqrt(var + EPS) * g + b


def _host_row0(inputs):
    """Exact f32 recompute of output row 0 (attention row 0 is zero-padded)."""
    x0 = np.asarray(inputs["x"], np.float32)[:, 0, :]          # [B, D]
    h1 = _ln_np(x0 + inputs["bo"][None, :], inputs["g1"], inputs["beta1"])
    f1 = np.maximum(h1 @ inputs["W1"] + inputs["b1"], 0.0)
    ffn = f1 @ inputs["W2"] + inputs["b2"]
    return _ln_np(h1 + ffn, inputs["g3"], inputs["beta3"])


def _fallback_np(inputs):
    x = np.asarray(inputs["x"], np.float32)
    q = x @ inputs["Wq"] + inputs["bq"]
    k = x @ inputs["Wk"] + inputs["bk"]
    v = x @ inputs["Wv"] + inputs["bv"]

    def heads(t):
        return t.reshape(B, S, H, DK).transpose(0, 2, 1, 3)
    q, k, v = heads(q), heads(k), heads(v)
    sc = np.einsum('bhqd,bhkd->bhqk', q, k).astype(np.float32) / 8.0
    i = np.arange(S)[:, None]
    j = np.arange(S)[None, :]
    sc = np.where((j < i)[None, None], sc, -1e32)
    sc -= sc.max(-1, keepdims=True)
    e = np.exp(sc)
    att = e / e.sum(-1, keepdims=True)
    att = att * (i > 0)[None, None]
    ctx = np.einsum('bhqk,bhkd->bhqd', att, v)
    concat = ctx.transpose(0, 2, 1, 3).reshape(B, S, D)
    h1 = _ln_np(x + concat @ inputs["Wo"] + inputs["bo"],
                inputs["g1"], inputs["beta1"])
    f1 = np.maximum(h1 @ inputs["W1"] + inputs["b1"], 0.0)
    out = _ln_np(h1 + f1 @ inputs["W2"] + inputs["b2"],
                 inputs["g3"], inputs["beta3"])
    return out.astype(np.float32)


def _fast_path_ok(inputs):
    z = lambda n: not np.any(np.asarray(inputs[n]))
    o = lambda n: np.all(np.asarray(inputs[n]) == 1.0)
    return (z("bq") and z("bk") and z("bv") and z("bo") and z("b1")
            and z("b2") and z("beta1") and z("beta3") and o("g1") and o("g3"))


def _split8(w):
    """fp8 hi + scaled lo decomposition of a weight matrix."""
    w = np.asarray(w, np.float32)
    hi = w.astype(NP_F8)
    lo8 = (8.0 * (w - hi.astype(np.float32))).astype(NP_F8)
    return hi, lo8


def kernel(**inputs):
    x = np.asarray(inputs["x"], dtype=np.float32)
    assert x.shape == (B, S, D)
    if not _fast_path_ok(inputs):
        return _fallback_np(inputs)

    f8 = lambda a: np.ascontiguousarray(np.asarray(a)).astype(NP_F8)
    # permute Wq/Wk columns so each 2-head group's 64 projection outputs
    # per d-half (heads 2g+u2, d-half i) are contiguous: col
    # 128g+64i+32u2+d32 maps to original head 2g+u2, d = 32i + d32.
    col = np.arange(D)
    g_, r_ = np.divmod(col, 128)
    i_, r2_ = np.divmod(r_, 64)
    u_, d_ = np.divmod(r2_, 32)
    perm = 64 * (2 * g_ + u_) + 32 * i_ + d_
    wvh, wvl = _split8(inputs["Wv"])
    woh, wol = _split8(inputs["Wo"])
    w1 = np.asarray(inputs["W1"], np.float32)
    w1t = np.ascontiguousarray(
        w1.reshape(DC, P, FC, P).transpose(2, 1, 0, 3))
    w1h = w1t.astype(NP_F8)
    w1l = (8.0 * (w1t - w1h.astype(np.float32))).astype(NP_F8)
    w2h, w2l = _split8(inputs["W2"])
    common = dict(
        wq8=f8(np.asarray(inputs["Wq"])[:, perm]),
        wk8=f8(np.asarray(inputs["Wk"])[:, perm]),
        wvh=wvh, wvl=wvl, woh=woh, wol=wol,
        w1h=w1h, w1l=w1l, w2h=w2h, w2l=w2l,
        mask8=np.triu(np.ones((P, P), np.float32), k=1).astype(NP_F8),
        ident=np.eye(P, dtype=np.float32),
        onesf=np.ones((P, 64), np.float32),
    )
    in_maps = []
    for i in range(B):
        xi = np.ascontiguousarray(x[i])
        xiT = np.ascontiguousarray(xi.T)
        xhi = xiT.astype(NP_F8)
        xlo = (xiT - xhi.astype(np.float32)).astype(NP_F8)
        xh8 = (xhi.astype(np.float32) / 8.0).astype(NP_F8)
        in_maps.append(dict(common, x32=xi, xT8=xhi, xTl8=xlo, xTh8=xh8))
    nc = _get_prog()
    res = run_bass_kernel_spmd(nc, in_maps, list(range(B)))
    out = np.stack([res.results[i]["out"] for i in range(B)], axis=0)
    out[:, 0, :] = _host_row0(inputs)
    return out
